# revision 2
# baseline (speedup 1.0000x reference)
"""Trainium2 Bass kernel for nn_BmmEnsemble (species-routed CELU-MLP ensemble).

v3 strategy (data-parallel over atoms, 8 NeuronCores):
  host: stable-sort atoms by species, shard species blocks across cores
        (C=1536 atoms/species/core, no padding), fp8-quantize aev,
        pack DoubleRow operands, pre-fold biases.
  device per tile-unit (species s, 512-atom tile t):
    L1: z+b+a in PSUM via fp8e4m3 DoubleRow matmuls (2 per 128-out chunk,
        K=386 = 384 aev rows + bias row + bias-residual row).
    drain (exact CELU, 2 passes):  celu(w)+a == max(w+a, min(a*e^{10 w}, a))
        ACT:  u' = Exp(10*psum + (ln a - 10 a)) -> bf16    [psum holds w+a]
        DVE:  h  = (u' MIN a) MAX psum          -> bf16 (scalar_tensor_tensor)
        +a shift folded into next layer's bias: b_eff = b - a*sum_fin(W) + a.
    L2/L3: f32r (exact) stationary weights x bf16 moving activations,
        f32r bias matmuls.
    readout: 8 accumulating W4 (f32r) matmuls -> e[1, 512] PSUM row,
        one DVE accum -> acc column per unit.
  Emission is software-pipelined: unit u's L1 matmuls are issued before
  unit u-1's L2/L3/readout so the PE never waits on drains.
  Host: subtract a*sum(W4) shift terms, add b4 terms, evaluate the
  4*212 leftover atoms exactly in f64.
"""
import math
import numpy as np
import ml_dtypes

BF16 = ml_dtypes.bfloat16
E4M3 = ml_dtypes.float8_e4m3fn if hasattr(ml_dtypes, 'float8_e4m3fn') else ml_dtypes.float8_e4m3

S = 4
M = 8
F0, F1, F2, F3 = 384, 160, 128, 96
ALPHA = 0.1
N_CORES = 8
T = 512
C = 1536
NT = C // T
N_UNITS = S * NT
LNAC = math.log(ALPHA) - 10.0 * ALPHA

# h1 tile block layout: drain-order blocks; chunk -> block
#   slot1 = chunks [8,9,0,1] -> blocks 0..3
#   slot2 = chunks [2,3,4,5] -> blocks 4..7
#   slot3 = chunks [6,7]     -> blocks 8..9
CHUNK_ORDER = [8, 9, 0, 1, 2, 3, 4, 5, 6, 7]
BLK = {c: i for i, c in enumerate(CHUNK_ORDER)}

_BUILD_CACHE = {}


def build_kernel():
    if "nc" in _BUILD_CACHE:
        return _BUILD_CACHE["nc"]

    import concourse.bacc as bacc
    import concourse.tile as tile
    import concourse.mybir as mybir

    F32 = mybir.dt.float32
    F32R = mybir.dt.float32r
    DBF = mybir.dt.bfloat16
    DE4 = mybir.dt.float8e4
    MIN, MAX, ADD = mybir.AluOpType.min, mybir.AluOpType.max, mybir.AluOpType.add
    EXP = mybir.ActivationFunctionType.Exp
    RELU = mybir.ActivationFunctionType.Relu
    DR = mybir.MatmulPerfMode.DoubleRow
    BANK = 512

    nc = bacc.Bacc("TRN2", target_bir_lowering=False, debug=False)

    xa_d = nc.dram_tensor("xa", [S, NT, 128, 2 * T], DE4, kind="ExternalInput").ap()
    xb_d = nc.dram_tensor("xb", [S, NT, 65, 2 * T], DE4, kind="ExternalInput").ap()
    w1a_d = nc.dram_tensor("w1a", [S, 128, 10 * 256], DE4, kind="ExternalInput").ap()
    w1b_d = nc.dram_tensor("w1b", [S, 65, 10 * 256], DE4, kind="ExternalInput").ap()
    w2m_d = nc.dram_tensor("w2m", [S, 128, M * 128], F32R, kind="ExternalInput").ap()
    w2r_d = nc.dram_tensor("w2r", [S, 128, 2 * 128], F32R, kind="ExternalInput").ap()
    w3_d = nc.dram_tensor("w3", [S, 128, M * 96], F32R, kind="ExternalInput").ap()
    w4_d = nc.dram_tensor("w4", [S, 96, M], F32R, kind="ExternalInput").ap()
    bl2_d = nc.dram_tensor("bl2", [S, 128, 256], F32R, kind="ExternalInput").ap()
    bl3_d = nc.dram_tensor("bl3", [S, 128, 192], F32R, kind="ExternalInput").ap()
    ones_d = nc.dram_tensor("ones", [128, T], F32R, kind="ExternalInput").ap()
    acc_d = nc.dram_tensor("acc", [1, N_UNITS], F32, kind="ExternalOutput").ap()

    with tile.TileContext(nc) as tc:
        with tc.tile_pool(name="wpool", bufs=1) as wpool, \
             tc.tile_pool(name="xpool", bufs=2) as xpool, \
             tc.tile_pool(name="hpool", bufs=2) as hpool, \
             tc.tile_pool(name="h2pool", bufs=1) as h2pool, \
             tc.tile_pool(name="upool", bufs=3) as upool, \
             tc.tile_pool(name="rpool", bufs=2) as rpool, \
             tc.tile_pool(name="apool", bufs=1) as apool, \
             tc.tile_pool(name="ps", bufs=4, space="PSUM") as psp:

            w1a_t, w1b_t, w2m_t, w2r_t, w3_t, w4_t, bl2_t, bl3_t = ({} for _ in range(8))
            for s in range(S):
                w1a_t[s] = wpool.tile([128, 10 * 256], DE4, tag=f"w1a{s}", name=f"w1a{s}")
                w1b_t[s] = wpool.tile([65, 10 * 256], DE4, tag=f"w1b{s}", name=f"w1b{s}")
                w2m_t[s] = wpool.tile([128, M * 128], F32R, tag=f"w2m{s}", name=f"w2m{s}")
                w2r_t[s] = wpool.tile([128, 2 * 128], F32R, tag=f"w2r{s}", name=f"w2r{s}")
                w3_t[s] = wpool.tile([128, M * 96], F32R, tag=f"w3{s}", name=f"w3{s}")
                w4_t[s] = wpool.tile([96, M], F32R, tag=f"w4{s}", name=f"w4{s}")
                bl2_t[s] = wpool.tile([128, 256], F32R, tag=f"bl2{s}", name=f"bl2{s}")
                bl3_t[s] = wpool.tile([128, 192], F32R, tag=f"bl3{s}", name=f"bl3{s}")

            def dma_l1_weights(s):
                nc.sync.dma_start(w1a_t[s][:, 0:4 * 256], w1a_d[s][:, 0:4 * 256])
                nc.sync.dma_start(w1b_t[s][:, 0:4 * 256], w1b_d[s][:, 0:4 * 256])
                nc.sync.dma_start(w1a_t[s][:, 4 * 256:], w1a_d[s][:, 4 * 256:])
                nc.sync.dma_start(w1b_t[s][:, 4 * 256:], w1b_d[s][:, 4 * 256:])

            def dma_rest_weights(s):
                for tt, dd in ((w2m_t[s], w2m_d[s]), (w2r_t[s], w2r_d[s]),
                               (w3_t[s], w3_d[s]), (w4_t[s], w4_d[s]),
                               (bl2_t[s], bl2_d[s]), (bl3_t[s], bl3_d[s])):
                    nc.sync.dma_start(tt[:], dd)

            ones_t = wpool.tile([128, T], F32R, tag="ones", name="ones")
            nc.sync.dma_start(ones_t[:], ones_d)
            lnac_t = wpool.tile([128, 1], F32, tag="lnac", name="lnac")
            nc.vector.memset(lnac_t[:], LNAC)
            acc_t = apool.tile([1, N_UNITS], F32, tag="acc", name="acc")
            junk_t = apool.tile([1, T], F32, tag="junk", name="junk")

            def drain(ps_t, nrow, gsz, h_view):
                ps_v = ps_t[0:nrow, :].rearrange("p (g q) -> p g q", q=BANK)[:, 0:gsz, 0:T]
                u = upool.tile([128, 2 * T], DBF, tag="u", name="u")
                u_v = u[0:nrow, 0:gsz * T].rearrange("p (g q) -> p g q", q=T)
                nc.scalar.activation(u_v, ps_v, EXP, bias=lnac_t[0:nrow, 0:1], scale=10.0)
                nc.vector.scalar_tensor_tensor(h_view, u_v, ALPHA, ps_v, op0=MIN, op1=MAX)

            def drain_p(ps_t, nrow, gsz, h_view):
                # Pool-assisted drain: ACT exp + ACT relu + Pool STT (no PSUM on Pool)
                ps_v = ps_t[0:nrow, :].rearrange("p (g q) -> p g q", q=BANK)[:, 0:gsz, 0:T]
                u = upool.tile([128, 2 * T], DBF, tag="u", name="u")
                u_v = u[0:nrow, 0:gsz * T].rearrange("p (g q) -> p g q", q=T)
                nc.scalar.activation(u_v, ps_v, EXP, bias=lnac_t[0:nrow, 0:1], scale=10.0)
                r = rpool.tile([128, 2 * T], DBF, tag="r", name="r")
                r_v = r[0:nrow, 0:gsz * T].rearrange("p (g q) -> p g q", q=T)
                nc.scalar.activation(r_v, ps_v, RELU, bias=0.0, scale=1.0)
                nc.gpsimd.scalar_tensor_tensor(h_view, u_v, ALPHA, r_v, op0=MIN, op1=MAX)

            def hview(h_t, b0, g, nrow=128):
                return h_t[0:nrow, b0 * T:(b0 + g) * T].rearrange("p (g q) -> p g q", q=T)

            def l1_slot_fns(s, xa_v, xb_v, h1):
                def mk(slot_i, chunks):
                    def fn():
                        ps_t = psp.tile([128, 2 * BANK], F32, tag="psg", name=f"l1s{slot_i}")
                        for g, c in enumerate(chunks):
                            k = BLK[c]  # host packs chunk CHUNK_ORDER[k] at col block k
                            nc.tensor.matmul(
                                ps_t[:, g * BANK:g * BANK + T],
                                w1a_t[s][:, k * 256:(k + 1) * 256].rearrange(
                                    "p (i m) -> p i m", i=2),
                                xa_v, start=True, stop=False, perf_mode=DR)
                            nc.tensor.matmul(
                                ps_t[:, g * BANK:g * BANK + T],
                                w1b_t[s][:, k * 256:(k + 1) * 256].rearrange(
                                    "p (i m) -> p i m", i=2),
                                xb_v, start=False, stop=True, perf_mode=DR)
                        drain(ps_t, 128, len(chunks),
                              hview(h1, BLK[chunks[0]], len(chunks)))
                    return fn
                return [mk(i, ch) for i, ch in
                        enumerate(([8, 9], [0, 1], [2, 3], [4, 5], [6, 7]))]

            def l2_slot_fns(s, h1, h2):
                def mk(m0):
                  def fn():
                    ps_t = psp.tile([128, 2 * BANK], F32, tag="psg", name=f"l2m{m0}")
                    for g in range(2):
                        m = m0 + g
                        reg, j = m // 4, m % 4
                        bm = BLK[m]
                        sl = slice(g * BANK, g * BANK + T)
                        nc.tensor.matmul(
                            ps_t[:, sl],
                            w2m_t[s][:, m * 128:(m + 1) * 128],
                            h1[:, bm * T:(bm + 1) * T],
                            start=True, stop=False)
                        nc.tensor.matmul(
                            ps_t[:, sl],
                            w2r_t[s][32 * j:32 * (j + 1), reg * 128:(reg + 1) * 128],
                            h1[32 * j:32 * (j + 1), BLK[8 + reg] * T:(BLK[8 + reg] + 1) * T],
                            start=False, stop=False, tile_position=(32 * j, 0))
                        nc.tensor.matmul(
                            ps_t[0:128, sl],
                            bl2_t[s][32 * j:32 * j + 1, reg * 128:reg * 128 + 128],
                            ones_t[32 * j:32 * j + 1, 0:T],
                            start=False, stop=True, tile_position=(32 * j, 0))
                    drain(ps_t, 128, 2, hview(h2, m0, 2))
                  return fn
                return [mk(m0) for m0 in (0, 2, 4, 6)]

            def l3_slot_fns(s, h2, h3):
                def mk(m0):
                  def fn():
                    ps_t = psp.tile([128, 2 * BANK], F32, tag="psg", name=f"l3m{m0}")
                    for g in range(2):
                        m = m0 + g
                        reg, j = m // 4, m % 4
                        sl = slice(g * BANK, g * BANK + T)
                        nc.tensor.matmul(
                            ps_t[0:96, sl],
                            w3_t[s][:, m * 96:(m + 1) * 96],
                            h2[:, m * T:(m + 1) * T],
                            start=True, stop=False)
                        nc.tensor.matmul(
                            ps_t[0:96, sl],
                            bl3_t[s][32 * j:32 * j + 1, reg * 96:reg * 96 + 96],
                            ones_t[32 * j:32 * j + 1, 0:T],
                            start=False, stop=True, tile_position=(32 * j, 0))
                    drain(ps_t, 96, 2, hview(h3, m0, 2, nrow=96))
                  return fn
                return [mk(m0) for m0 in (0, 2, 4, 6)]

            def readout_fn(s, unit, h3):
              def fn():
                ps_t = psp.tile([128, 2 * BANK], F32, tag="psg", name="eps")
                for m in range(M):
                    nc.tensor.matmul(
                        ps_t[0:1, 0:T],
                        w4_t[s][:, m:m + 1],
                        h3[:, m * T:(m + 1) * T],
                        start=(m == 0), stop=(m == M - 1))
                nc.scalar.activation(
                    junk_t[:], ps_t[0:1, 0:T],
                    mybir.ActivationFunctionType.Identity,
                    bias=0.0, scale=1.0,
                    accum_out=acc_t[0:1, unit:unit + 1])
              return fn

            units = [(s, t) for s in range(S) for t in range(NT)]
            pending = None
            dma_l1_weights(0)
            for unit, (s, t) in enumerate(units):
                xa_t = xpool.tile([128, 2 * T], DE4, tag="xa", name="xa")
                xb_t = xpool.tile([65, 2 * T], DE4, tag="xb", name="xb")
                nc.sync.dma_start(xa_t[:], xa_d[s, t])
                nc.sync.dma_start(xb_t[:], xb_d[s, t])
                if t == 0:
                    dma_rest_weights(s)
                if t == NT - 1 and s + 1 < S:
                    dma_l1_weights(s + 1)
                h1 = hpool.tile([128, 10 * T], F32R, tag="h1", name="h1")
                h2 = h2pool.tile([128, 8 * T], F32R, tag="h2", name="h2")
                h3 = h2pool.tile([96, 8 * T], F32R, tag="h3", name="h3")
                l1f = l1_slot_fns(s, xa_t[:].rearrange("p (i q) -> p i q", i=2),
                                  xb_t[:].rearrange("p (i q) -> p i q", i=2), h1)
                s2f = []
                if pending is not None:
                    us_, uu_, h1_, h2_, h3_ = pending
                    s2f = (l2_slot_fns(us_, h1_, h2_) + l3_slot_fns(us_, h2_, h3_)
                           + [readout_fn(us_, uu_, h3_)])
                # interleave ~1 L1 slot per 2 stage-2 slots, L1 first
                if s2f:
                    order = [l1f[0], s2f[0], s2f[1], l1f[1], s2f[2], s2f[3],
                             l1f[2], s2f[4], s2f[5], s2f[6], s2f[7],
                             l1f[3], l1f[4], s2f[8]]
                else:
                    order = l1f
                for fn in order:
                    fn()
                pending = (s, unit, h1, h2, h3)
            us_, uu_, h1_, h2_, h3_ = pending
            for fn in (l2_slot_fns(us_, h1_, h2_) + l3_slot_fns(us_, h2_, h3_)
                       + [readout_fn(us_, uu_, h3_)]):
                fn()

            nc.sync.dma_start(acc_d, acc_t[:])

    nc.compile()
    _BUILD_CACHE["nc"] = nc
    return nc


# ----------------------------------------------------------------------------
# host-side packing
# ----------------------------------------------------------------------------
def _celu64(x):
    return np.where(x > 0, x, ALPHA * np.expm1(np.minimum(x, 0) / ALPHA))


def _q(x, dt):
    return np.asarray(x).astype(dt).astype(np.float64)


def prep_inputs(species, aev, W1, b1, W2, b2, W3, b3, W4, b4):
    sp = np.asarray(species).reshape(-1)
    n_atoms = sp.shape[0]
    aev0 = np.asarray(aev, dtype=np.float32).reshape(n_atoms, F0)
    W1, b1, W2, b2, W3, b3, W4, b4 = [np.asarray(a, np.float64) for a in
                                      (W1, b1, W2, b2, W3, b3, W4, b4)]

    order = np.argsort(sp, kind="stable")
    cnt = np.bincount(sp.astype(np.int64), minlength=S)
    starts = np.concatenate([[0], np.cumsum(cnt)])
    dev_cnt = np.minimum(cnt, N_CORES * C)
    assert int(cnt.min()) >= N_CORES * C, "expected >= 12288 atoms per species"

    aev8 = aev0.astype(E4M3).astype(np.float32)

    xas = [np.zeros((S, NT, 128, 2 * T), dtype=E4M3) for _ in range(N_CORES)]
    xbs = [np.zeros((S, NT, 65, 2 * T), dtype=E4M3) for _ in range(N_CORES)]
    for s in range(S):
        idx = order[starts[s]:starts[s] + dev_cnt[s]]
        blk = aev8[idx]
        for c in range(N_CORES):
            seg = blk[c * C:(c + 1) * C]
            xf = np.concatenate([seg.T, np.ones((2, C), np.float32)], axis=0)
            for t in range(NT):
                col = xf[:, t * T:(t + 1) * T]
                xas[c][s, t] = col[0:256].reshape(128, 2 * T).astype(E4M3)
                xbs[c][s, t] = col[256:386].reshape(65, 2 * T).astype(E4M3)

    # W1 DR-packed with bias + bias-residual rows (fp8)
    w1a = np.zeros((S, 128, 10 * 256), dtype=E4M3)
    w1b = np.zeros((S, 65, 10 * 256), dtype=E4M3)
    for s in range(S):
        cols = np.zeros((F0, 1280), np.float64)
        brow = np.zeros(1280, np.float64)
        for c in range(8):
            cols[:, 128 * c:128 * (c + 1)] = W1[s, c, :, 0:128]
            brow[128 * c:128 * (c + 1)] = b1[s, c, 0, 0:128] + ALPHA
        for r in range(2):
            for j in range(4):
                c0 = 1024 + 128 * r + 32 * j
                cols[:, c0:c0 + 32] = W1[s, 4 * r + j, :, 128:160]
                brow[c0:c0 + 32] = b1[s, 4 * r + j, 0, 128:160] + ALPHA
        b_hi = _q(brow, E4M3)
        b_lo = brow - b_hi
        full = np.concatenate([cols, b_hi[None, :], b_lo[None, :]], axis=0)
        fq = full.astype(np.float32).astype(E4M3)
        for k, c in enumerate(CHUNK_ORDER):
            blkc = fq[:, 128 * c:128 * (c + 1)].astype(np.float32)
            w1a[s][:, k * 256:(k + 1) * 256] = blkc[0:256].reshape(128, 256).astype(E4M3)
            w1b[s][:, k * 256:(k + 1) * 256] = blkc[256:386].reshape(65, 256).astype(E4M3)

    w2m = np.zeros((S, 128, M * 128), np.float32)
    w2r = np.zeros((S, 128, 2 * 128), np.float32)
    bl2 = np.zeros((S, 128, 256), np.float32)
    for s in range(S):
        for m in range(M):
            reg, j = m // 4, m % 4
            w2m[s][:, m * 128:(m + 1) * 128] = W2[s, m, 0:128]
            w2r[s][32 * j:32 * (j + 1), reg * 128:(reg + 1) * 128] = W2[s, m, 128:160]
            beff = b2[s, m, 0, :] - ALPHA * W2[s, m].sum(axis=0) + ALPHA
            bl2[s][32 * j, reg * 128:reg * 128 + 128] = beff.astype(np.float32)

    w3p = np.zeros((S, 128, M * 96), np.float32)
    bl3 = np.zeros((S, 128, 192), np.float32)
    for s in range(S):
        for m in range(M):
            reg, j = m // 4, m % 4
            w3p[s][:, m * 96:(m + 1) * 96] = W3[s, m]
            beff = b3[s, m, 0, :] - ALPHA * W3[s, m].sum(axis=0) + ALPHA
            bl3[s][32 * j, reg * 96:reg * 96 + 96] = beff.astype(np.float32)

    w4p = np.zeros((S, 96, M), np.float32)
    for s in range(S):
        for m in range(M):
            w4p[s][:, m] = W4[s, m, :, 0].astype(np.float32)

    common = {"w1a": w1a, "w1b": w1b, "w2m": w2m, "w2r": w2r, "w3": w3p,
              "w4": w4p, "bl2": bl2, "bl3": bl3,
              "ones": np.ones((128, T), np.float32)}
    in_maps = [dict(common, xa=xas[c], xb=xbs[c]) for c in range(N_CORES)]

    w4sum = np.array([_q(W4[s, :, :, 0].astype(np.float32), np.float64).sum()
                      for s in range(S)])
    b4sum = b4[:, :, 0, 0].sum(axis=1)

    leftover = 0.0
    for s in range(S):
        n_left = int(cnt[s] - dev_cnt[s])
        if n_left <= 0:
            continue
        idx = order[starts[s] + dev_cnt[s]:starts[s + 1]]
        x = aev0[idx].astype(np.float64)
        for m in range(M):
            h = _celu64(x @ W1[s, m] + b1[s, m, 0])
            h = _celu64(h @ W2[s, m] + b2[s, m, 0])
            h = _celu64(h @ W3[s, m] + b3[s, m, 0])
            leftover += float((h @ W4[s, m, :, 0]).sum()) + n_left * float(b4[s, m, 0, 0])

    def finish(results):
        tot = 0.0
        for res in results:
            a = res["acc"].astype(np.float64).reshape(N_UNITS)
            for s in range(S):
                for t in range(NT):
                    tot += a[s * NT + t] - T * ALPHA * w4sum[s]
        for s in range(S):
            tot += dev_cnt[s] * b4sum[s]
        tot += leftover
        return np.array([tot / M], dtype=np.float32)

    return in_maps, finish


def _ensure_axon_platform():
    try:
        import jax
        devs = jax.devices()
        if len(devs) >= N_CORES and devs[0].platform != "cpu":
            return
        jax.config.update("jax_platforms", "axon")
    except Exception:
        pass


def kernel(**inputs):
    from concourse.bass_utils import run_bass_kernel_spmd
    _ensure_axon_platform()
    in_maps, finish = prep_inputs(**inputs)
    nc = build_kernel()
    res = run_bass_kernel_spmd(nc, in_maps, list(range(N_CORES)))
    return finish(res.results)


# revision 5
# speedup vs baseline: 1.0522x; 1.0522x over previous
"""Trainium2 Bass kernel for nn_BmmEnsemble (species-routed CELU-MLP ensemble).

v3 strategy (data-parallel over atoms, 8 NeuronCores):
  host: stable-sort atoms by species, shard species blocks across cores
        (C=1536 atoms/species/core, no padding), fp8-quantize aev,
        pack DoubleRow operands, pre-fold biases.
  device per tile-unit (species s, 512-atom tile t):
    L1: z+b+a in PSUM via fp8e4m3 DoubleRow matmuls (2 per 128-out chunk,
        K=386 = 384 aev rows + bias row + bias-residual row).
    drain (exact CELU, 2 passes):  celu(w)+a == max(w+a, min(a*e^{10 w}, a))
        ACT:  u' = Exp(10*psum + (ln a - 10 a)) -> bf16    [psum holds w+a]
        DVE:  h  = (u' MIN a) MAX psum          -> bf16 (scalar_tensor_tensor)
        +a shift folded into next layer's bias: b_eff = b - a*sum_fin(W) + a.
    L2/L3: f32r (exact) stationary weights x bf16 moving activations,
        f32r bias matmuls.
    readout: 8 accumulating W4 (f32r) matmuls -> e[1, 512] PSUM row,
        one DVE accum -> acc column per unit.
  Emission is software-pipelined: unit u's L1 matmuls are issued before
  unit u-1's L2/L3/readout so the PE never waits on drains.
  Host: subtract a*sum(W4) shift terms, add b4 terms, evaluate the
  4*212 leftover atoms exactly in f64.
"""
import math
import numpy as np
import ml_dtypes

BF16 = ml_dtypes.bfloat16
E4M3 = ml_dtypes.float8_e4m3fn if hasattr(ml_dtypes, 'float8_e4m3fn') else ml_dtypes.float8_e4m3

S = 4
M = 8
F0, F1, F2, F3 = 384, 160, 128, 96
ALPHA = 0.1
N_CORES = 8
T = 512
C_MAX = 1536
LNAC = math.log(ALPHA) - 10.0 * ALPHA

# h1 tile block layout: drain-order blocks; chunk -> block
#   slot1 = chunks [8,9,0,1] -> blocks 0..3
#   slot2 = chunks [2,3,4,5] -> blocks 4..7
#   slot3 = chunks [6,7]     -> blocks 8..9
CHUNK_ORDER = [8, 9, 0, 1, 2, 3, 4, 5, 6, 7]
BLK = {c: i for i, c in enumerate(CHUNK_ORDER)}

# L3 output packing: 8 models x 96 feats -> 6 full 128-partition banks.
# Pieces per 4-model group g (base model 4g, base bank 3g):
#   (model, f0, f1, bank, partition offset)
L3_PIECES = []
for _g in (0, 1):
    _M, _B = 4 * _g, 3 * _g
    L3_PIECES += [
        (_M + 0, 0, 96, _B + 0, 0),
        (_M + 1, 0, 32, _B + 0, 96),
        (_M + 1, 32, 96, _B + 1, 0),
        (_M + 2, 0, 64, _B + 1, 64),
        (_M + 2, 64, 96, _B + 2, 0),
        (_M + 3, 0, 32, _B + 2, 32),
        (_M + 3, 32, 64, _B + 2, 64),
        (_M + 3, 64, 96, _B + 2, 96),
    ]
# piece -> column offset in the packed w3 tile (piece-major)
L3_PCOL = []
_off = 0
for (_m, _f0, _f1, _b, _o) in L3_PIECES:
    L3_PCOL.append(_off)
    _off += _f1 - _f0

_BUILD_CACHE = {}


def build_kernel(C=C_MAX):
    if C in _BUILD_CACHE:
        return _BUILD_CACHE[C]
    NT = C // T
    N_UNITS = S * NT

    import concourse.bacc as bacc
    import concourse.tile as tile
    import concourse.mybir as mybir

    F32 = mybir.dt.float32
    F32R = mybir.dt.float32r
    DBF = mybir.dt.bfloat16
    DF16 = mybir.dt.float16
    DE4 = mybir.dt.float8e4
    MIN, MAX, ADD = mybir.AluOpType.min, mybir.AluOpType.max, mybir.AluOpType.add
    EXP = mybir.ActivationFunctionType.Exp
    RELU = mybir.ActivationFunctionType.Relu
    DR = mybir.MatmulPerfMode.DoubleRow
    BANK = 512

    nc = bacc.Bacc("TRN2", target_bir_lowering=False, debug=False)

    xa_d = nc.dram_tensor("xa", [S, NT, 128, 2 * T], DE4, kind="ExternalInput").ap()
    xb_d = nc.dram_tensor("xb", [S, NT, 65, 2 * T], DE4, kind="ExternalInput").ap()
    w1a_d = nc.dram_tensor("w1a", [S, 128, 10 * 256], DE4, kind="ExternalInput").ap()
    w1b_d = nc.dram_tensor("w1b", [S, 65, 10 * 256], DE4, kind="ExternalInput").ap()
    w2m_d = nc.dram_tensor("w2m", [S, 128, M * 128], F32R, kind="ExternalInput").ap()
    w2r_d = nc.dram_tensor("w2r", [S, 128, 2 * 128], F32R, kind="ExternalInput").ap()
    w3_d = nc.dram_tensor("w3", [S, 128, M * 96], DF16, kind="ExternalInput").ap()
    w4_d = nc.dram_tensor("w4", [S, 128, 6], F32R, kind="ExternalInput").ap()
    bl2_d = nc.dram_tensor("bl2", [S, 128, 256], F32R, kind="ExternalInput").ap()
    bl3_d = nc.dram_tensor("bl3", [S, 128, 256], F32R, kind="ExternalInput").ap()
    ones_d = nc.dram_tensor("ones", [128, T], F32R, kind="ExternalInput").ap()
    acc_d = nc.dram_tensor("acc", [1, N_UNITS], F32, kind="ExternalOutput").ap()

    with tile.TileContext(nc) as tc:
        with tc.tile_pool(name="wpool", bufs=1) as wpool, \
             tc.tile_pool(name="xpool", bufs=2) as xpool, \
             tc.tile_pool(name="hpool", bufs=2) as hpool, \
             tc.tile_pool(name="h2pool", bufs=1) as h2pool, \
             tc.tile_pool(name="upool", bufs=3) as upool, \
             tc.tile_pool(name="apool", bufs=1) as apool, \
             tc.tile_pool(name="ps", bufs=4, space="PSUM") as psp:

            w1a_t, w1b_t, w2m_t, w2r_t, w3_t, w4_t, bl2_t, bl3_t = ({} for _ in range(8))
            for s in range(S):
                w1a_t[s] = wpool.tile([128, 10 * 256], DE4, tag=f"w1a{s}", name=f"w1a{s}")
                w1b_t[s] = wpool.tile([65, 10 * 256], DE4, tag=f"w1b{s}", name=f"w1b{s}")
                w2m_t[s] = wpool.tile([128, M * 128], F32R, tag=f"w2m{s}", name=f"w2m{s}")
                w2r_t[s] = wpool.tile([128, 2 * 128], F32R, tag=f"w2r{s}", name=f"w2r{s}")
                w3_t[s] = wpool.tile([128, M * 96], DF16, tag=f"w3{s}", name=f"w3{s}")
                w4_t[s] = wpool.tile([128, 6], F32R, tag=f"w4{s}", name=f"w4{s}")
                bl2_t[s] = wpool.tile([128, 256], F32R, tag=f"bl2{s}", name=f"bl2{s}")
                bl3_t[s] = wpool.tile([128, 256], F32R, tag=f"bl3{s}", name=f"bl3{s}")

            def dma_l1_weights(s, split=False):
                nc.sync.dma_start(w1a_t[s][:, 0:3 * 256], w1a_d[s][:, 0:3 * 256])
                nc.sync.dma_start(w1b_t[s][:, 0:3 * 256], w1b_d[s][:, 0:3 * 256])
                if not split:
                    nc.sync.dma_start(w1a_t[s][:, 3 * 256:], w1a_d[s][:, 3 * 256:])
                    nc.sync.dma_start(w1b_t[s][:, 3 * 256:], w1b_d[s][:, 3 * 256:])

            def dma_l1_weights_rest(s):
                nc.sync.dma_start(w1a_t[s][:, 3 * 256:], w1a_d[s][:, 3 * 256:])
                nc.sync.dma_start(w1b_t[s][:, 3 * 256:], w1b_d[s][:, 3 * 256:])

            def dma_rest_weights(s):
                for tt, dd in ((w2m_t[s], w2m_d[s]), (w2r_t[s], w2r_d[s]),
                               (w3_t[s], w3_d[s]), (w4_t[s], w4_d[s]),
                               (bl2_t[s], bl2_d[s]), (bl3_t[s], bl3_d[s])):
                    nc.sync.dma_start(tt[:], dd)

            ones_t = wpool.tile([128, T], F32R, tag="ones", name="ones")
            nc.sync.dma_start(ones_t[:], ones_d)
            lnac_t = wpool.tile([128, 1], F32, tag="lnac", name="lnac")
            nc.vector.memset(lnac_t[:], LNAC)
            # warm the ACT Exp table during the initial DMA window
            warm_t = wpool.tile([128, 1], F32, tag="warm", name="warm")
            nc.scalar.activation(warm_t[:], lnac_t[:],
                                 mybir.ActivationFunctionType.Exp,
                                 bias=0.0, scale=1.0)
            acc_t = apool.tile([1, N_UNITS], F32, tag="acc", name="acc")
            junk_t = apool.tile([1, T], F32, tag="junk", name="junk")

            def drain(ps_t, nrow, gsz, h_view):
                ps_v = ps_t[0:nrow, :].rearrange("p (g q) -> p g q", q=BANK)[:, 0:gsz, 0:T]
                u = upool.tile([128, 2 * T], DBF, tag="u", name="u")
                u_v = u[0:nrow, 0:gsz * T].rearrange("p (g q) -> p g q", q=T)
                nc.scalar.activation(u_v, ps_v, EXP, bias=lnac_t[0:nrow, 0:1], scale=10.0)
                nc.vector.scalar_tensor_tensor(h_view, u_v, ALPHA, ps_v, op0=MIN, op1=MAX)

            def hview(h_t, b0, g, nrow=128):
                return h_t[0:nrow, b0 * T:(b0 + g) * T].rearrange("p (g q) -> p g q", q=T)

            def l1_slot_fns(s, xa_v, xb_v, h1):
                def mk(slot_i, chunks):
                    def fn():
                        ps_t = psp.tile([128, 2 * BANK], F32, tag="psg", name=f"l1s{slot_i}")
                        for g, c in enumerate(chunks):
                            k = BLK[c]  # host packs chunk CHUNK_ORDER[k] at col block k
                            nc.tensor.matmul(
                                ps_t[:, g * BANK:g * BANK + T],
                                w1a_t[s][:, k * 256:(k + 1) * 256].rearrange(
                                    "p (i m) -> p i m", i=2),
                                xa_v, start=True, stop=False, perf_mode=DR)
                            nc.tensor.matmul(
                                ps_t[:, g * BANK:g * BANK + T],
                                w1b_t[s][:, k * 256:(k + 1) * 256].rearrange(
                                    "p (i m) -> p i m", i=2),
                                xb_v, start=False, stop=True, perf_mode=DR)
                        drain(ps_t, 128, len(chunks),
                              hview(h1, BLK[chunks[0]], len(chunks)))
                    return fn
                return [mk(i, ch) for i, ch in
                        enumerate(([8, 9], [0, 1], [2, 3], [4, 5], [6, 7]))]

            def l2_slot_fns(s, h1, h2):
                def mk(m0):
                  def fn():
                    ps_t = psp.tile([128, 2 * BANK], F32, tag="psg", name=f"l2m{m0}")
                    for g in range(2):
                        m = m0 + g
                        reg, j = m // 4, m % 4
                        bm = BLK[m]
                        sl = slice(g * BANK, g * BANK + T)
                        nc.tensor.matmul(
                            ps_t[:, sl],
                            w2m_t[s][:, m * 128:(m + 1) * 128],
                            h1[:, bm * T:(bm + 1) * T],
                            start=True, stop=False)
                        nc.tensor.matmul(
                            ps_t[:, sl],
                            w2r_t[s][32 * j:32 * (j + 1), reg * 128:(reg + 1) * 128],
                            h1[32 * j:32 * (j + 1), BLK[8 + reg] * T:(BLK[8 + reg] + 1) * T],
                            start=False, stop=False, tile_position=(32 * j, 0))
                        nc.tensor.matmul(
                            ps_t[0:128, sl],
                            bl2_t[s][32 * j:32 * j + 1, reg * 128:reg * 128 + 128],
                            ones_t[32 * j:32 * j + 1, 0:T],
                            start=False, stop=True, tile_position=(32 * j, 0))
                    drain(ps_t, 128, 2, hview(h2, m0, 2))
                  return fn
                return [mk(m0) for m0 in (0, 2, 4, 6)]

            def l3_slot_fns(s, h2, h3):
                # packed L3: 6 full banks, pieces per L3_PIECES
                def mk(b0):
                  def fn():
                    ps_t = psp.tile([128, 2 * BANK], F32, tag="psg", name=f"l3b{b0}")
                    for g in range(2):
                        bank = b0 + g
                        sl = slice(g * BANK, g * BANK + T)
                        for pi, (m, f0, f1, bk, o) in enumerate(L3_PIECES):
                            if bk != bank:
                                continue
                            w = f1 - f0
                            # pieces cover disjoint partition ranges: each
                            # must open its own accumulation region
                            nc.tensor.matmul(
                                ps_t[o:o + w, sl],
                                w3_t[s][:, L3_PCOL[pi]:L3_PCOL[pi] + w],
                                h2[:, m * T:(m + 1) * T],
                                start=True, stop=False,
                                tile_position=(0, o))
                        j, blk = bank % 4, bank // 4
                        nc.tensor.matmul(
                            ps_t[0:128, sl],
                            bl3_t[s][32 * j:32 * j + 1, blk * 128:blk * 128 + 128],
                            ones_t[32 * j:32 * j + 1, 0:T],
                            start=False, stop=True, tile_position=(32 * j, 0))
                    drain(ps_t, 128, 2, hview(h3, b0, 2))
                  return fn
                return [mk(b0) for b0 in (0, 2, 4)]

            def readout_fn(s, unit, h3):
              def fn():
                ps_t = psp.tile([128, 2 * BANK], F32, tag="psg", name="eps")
                for k in range(6):
                    nc.tensor.matmul(
                        ps_t[0:1, 0:T],
                        w4_t[s][:, k:k + 1],
                        h3[:, k * T:(k + 1) * T],
                        start=(k == 0), stop=(k == 5))
                nc.scalar.activation(
                    junk_t[:], ps_t[0:1, 0:T],
                    mybir.ActivationFunctionType.Identity,
                    bias=0.0, scale=1.0,
                    accum_out=acc_t[0:1, unit:unit + 1])
              return fn

            units = [(s, t) for s in range(S) for t in range(NT)]
            pending = None
            dma_l1_weights(0, split=True)
            for unit, (s, t) in enumerate(units):
                xa_t = xpool.tile([128, 2 * T], DE4, tag="xa", name="xa")
                xb_t = xpool.tile([65, 2 * T], DE4, tag="xb", name="xb")
                nc.sync.dma_start(xa_t[:], xa_d[s, t])
                nc.sync.dma_start(xb_t[:], xb_d[s, t])
                if unit == 0:
                    dma_l1_weights_rest(0)
                if t == 0:
                    dma_rest_weights(s)
                if t == NT - 1 and s + 1 < S:
                    dma_l1_weights(s + 1)
                h1 = hpool.tile([128, 10 * T], F32R, tag="h1", name="h1")
                h2 = h2pool.tile([128, 8 * T], DF16, tag="h2", name="h2")
                h3 = h2pool.tile([128, 6 * T], F32R, tag="h3", name="h3")
                l1f = l1_slot_fns(s, xa_t[:].rearrange("p (i q) -> p i q", i=2),
                                  xb_t[:].rearrange("p (i q) -> p i q", i=2), h1)
                s2f = []
                if pending is not None:
                    us_, uu_, h1_, h2_, h3_ = pending
                    s2f = (l2_slot_fns(us_, h1_, h2_) + l3_slot_fns(us_, h2_, h3_)
                           + [readout_fn(us_, uu_, h3_)])
                # interleave ~1 L1 slot per 2 stage-2 slots, L1 first
                if s2f:
                    order = [l1f[0], s2f[0], s2f[1], l1f[1], s2f[2], s2f[3],
                             l1f[2], s2f[4], s2f[5], s2f[6],
                             l1f[3], l1f[4], s2f[7]]
                else:
                    order = l1f
                for fn in order:
                    fn()
                pending = (s, unit, h1, h2, h3)
            us_, uu_, h1_, h2_, h3_ = pending
            for fn in (l2_slot_fns(us_, h1_, h2_) + l3_slot_fns(us_, h2_, h3_)
                       + [readout_fn(us_, uu_, h3_)]):
                fn()

            nc.sync.dma_start(acc_d, acc_t[:])

    nc.compile()
    _BUILD_CACHE[C] = nc
    return nc


# ----------------------------------------------------------------------------
# host-side packing
# ----------------------------------------------------------------------------
def _celu64(x):
    return np.where(x > 0, x, ALPHA * np.expm1(np.minimum(x, 0) / ALPHA))


def _q(x, dt):
    return np.asarray(x).astype(dt).astype(np.float64)


def prep_inputs(species, aev, W1, b1, W2, b2, W3, b3, W4, b4):
    sp = np.asarray(species).reshape(-1)
    n_atoms = sp.shape[0]
    aev0 = np.asarray(aev, dtype=np.float32).reshape(n_atoms, F0)
    W1, b1, W2, b2, W3, b3, W4, b4 = [np.asarray(a, np.float64) for a in
                                      (W1, b1, W2, b2, W3, b3, W4, b4)]

    order = np.argsort(sp, kind="stable")
    cnt = np.bincount(sp.astype(np.int64), minlength=S)
    starts = np.concatenate([[0], np.cumsum(cnt)])
    # device capacity: largest tile-multiple that every species fills exactly
    # (no padding on device); overflow atoms are evaluated on the host in f64
    C = min(C_MAX, (int(cnt.min()) // (N_CORES * T)) * T)
    assert C >= T, "species too unbalanced for device path"
    NT = C // T
    N_UNITS = S * NT
    dev_cnt = np.minimum(cnt, N_CORES * C)

    aev8 = aev0.astype(E4M3).astype(np.float32)

    xas = [np.zeros((S, NT, 128, 2 * T), dtype=E4M3) for _ in range(N_CORES)]
    xbs = [np.zeros((S, NT, 65, 2 * T), dtype=E4M3) for _ in range(N_CORES)]
    for s in range(S):
        idx = order[starts[s]:starts[s] + dev_cnt[s]]
        blk = aev8[idx]
        for c in range(N_CORES):
            seg = blk[c * C:(c + 1) * C]
            xf = np.concatenate([seg.T, np.ones((2, C), np.float32)], axis=0)
            for t in range(NT):
                col = xf[:, t * T:(t + 1) * T]
                xas[c][s, t] = col[0:256].reshape(128, 2 * T).astype(E4M3)
                xbs[c][s, t] = col[256:386].reshape(65, 2 * T).astype(E4M3)

    # W1 DR-packed with bias + bias-residual rows (fp8)
    w1a = np.zeros((S, 128, 10 * 256), dtype=E4M3)
    w1b = np.zeros((S, 65, 10 * 256), dtype=E4M3)
    for s in range(S):
        cols = np.zeros((F0, 1280), np.float64)
        brow = np.zeros(1280, np.float64)
        for c in range(8):
            cols[:, 128 * c:128 * (c + 1)] = W1[s, c, :, 0:128]
            brow[128 * c:128 * (c + 1)] = b1[s, c, 0, 0:128] + ALPHA
        for r in range(2):
            for j in range(4):
                c0 = 1024 + 128 * r + 32 * j
                cols[:, c0:c0 + 32] = W1[s, 4 * r + j, :, 128:160]
                brow[c0:c0 + 32] = b1[s, 4 * r + j, 0, 128:160] + ALPHA
        b_hi = _q(brow, E4M3)
        b_lo = brow - b_hi
        full = np.concatenate([cols, b_hi[None, :], b_lo[None, :]], axis=0)
        fq = full.astype(np.float32).astype(E4M3)
        for k, c in enumerate(CHUNK_ORDER):
            blkc = fq[:, 128 * c:128 * (c + 1)].astype(np.float32)
            w1a[s][:, k * 256:(k + 1) * 256] = blkc[0:256].reshape(128, 256).astype(E4M3)
            w1b[s][:, k * 256:(k + 1) * 256] = blkc[256:386].reshape(65, 256).astype(E4M3)

    w2m = np.zeros((S, 128, M * 128), np.float32)
    w2r = np.zeros((S, 128, 2 * 128), np.float32)
    bl2 = np.zeros((S, 128, 256), np.float32)
    for s in range(S):
        for m in range(M):
            reg, j = m // 4, m % 4
            w2m[s][:, m * 128:(m + 1) * 128] = W2[s, m, 0:128]
            w2r[s][32 * j:32 * (j + 1), reg * 128:(reg + 1) * 128] = W2[s, m, 128:160]
            beff = b2[s, m, 0, :] - ALPHA * W2[s, m].sum(axis=0) + ALPHA
            bl2[s][32 * j, reg * 128:reg * 128 + 128] = beff.astype(np.float32)

    w3p = np.zeros((S, 128, M * 96), dtype=np.float16)
    bl3 = np.zeros((S, 128, 256), np.float32)
    w4p = np.zeros((S, 128, 6), np.float32)
    for s in range(S):
        W3q = _q(W3[s].astype(np.float32), np.float16)     # device-quantized W3
        beff3 = np.stack([b3[s, m, 0, :] - ALPHA * W3q[m].sum(axis=0) + ALPHA
                          for m in range(M)])          # [M, 96]
        for pi, (m, f0, f1, bank, o) in enumerate(L3_PIECES):
            w = f1 - f0
            w3p[s][:, L3_PCOL[pi]:L3_PCOL[pi] + w] = W3q[m, :, f0:f1].astype(np.float16)
            j, blk = bank % 4, bank // 4
            bl3[s][32 * j, blk * 128 + o:blk * 128 + o + w] = \
                beff3[m, f0:f1].astype(np.float32)
            w4p[s][o:o + w, bank] = W4[s, m, f0:f1, 0].astype(np.float32)

    common = {"w1a": w1a, "w1b": w1b, "w2m": w2m, "w2r": w2r, "w3": w3p,
              "w4": w4p, "bl2": bl2, "bl3": bl3,
              "ones": np.ones((128, T), np.float32)}
    in_maps = [dict(common, xa=xas[c], xb=xbs[c]) for c in range(N_CORES)]

    w4sum = np.array([_q(W4[s, :, :, 0].astype(np.float32), np.float64).sum()
                      for s in range(S)])
    b4sum = b4[:, :, 0, 0].sum(axis=1)

    leftover = 0.0
    for s in range(S):
        n_left = int(cnt[s] - dev_cnt[s])
        if n_left <= 0:
            continue
        idx = order[starts[s] + dev_cnt[s]:starts[s + 1]]
        x = aev0[idx].astype(np.float64)
        for m in range(M):
            h = _celu64(x @ W1[s, m] + b1[s, m, 0])
            h = _celu64(h @ W2[s, m] + b2[s, m, 0])
            h = _celu64(h @ W3[s, m] + b3[s, m, 0])
            leftover += float((h @ W4[s, m, :, 0]).sum()) + n_left * float(b4[s, m, 0, 0])

    def finish(results):
        tot = 0.0
        for res in results:
            a = res["acc"].astype(np.float64).reshape(N_UNITS)
            for s in range(S):
                for t in range(NT):
                    tot += a[s * NT + t] - T * ALPHA * w4sum[s]
        for s in range(S):
            tot += dev_cnt[s] * b4sum[s]
        tot += leftover
        return np.array([tot / M], dtype=np.float32)

    return C, in_maps, finish


def _ensure_axon_platform():
    try:
        import jax
        devs = jax.devices()
        if len(devs) >= N_CORES and devs[0].platform != "cpu":
            return
        jax.config.update("jax_platforms", "axon")
    except Exception:
        pass


def kernel(**inputs):
    from concourse.bass_utils import run_bass_kernel_spmd
    _ensure_axon_platform()
    C, in_maps, finish = prep_inputs(**inputs)
    nc = build_kernel(C)
    res = run_bass_kernel_spmd(nc, in_maps, list(range(N_CORES)))
    return finish(res.results)


# revision 6
# speedup vs baseline: 1.0570x; 1.0046x over previous
"""Trainium2 Bass kernel for nn_BmmEnsemble (species-routed CELU-MLP ensemble).

Strategy (data-parallel over atoms, 8 NeuronCores):
  host: stable-sort atoms by species, shard species blocks across cores
        (C=1536 atoms/species/core, no padding), fp8-quantize aev,
        pack DoubleRow operands, pre-fold biases.
  device per tile-unit (species s, 512-atom tile t):
    L1: z+b+a in PSUM via fp8e4m3 DoubleRow matmuls (2 per 128-out chunk,
        K=386 = 384 aev rows + fp8 bias row + fp8 bias-residual row).
    drain (exact CELU, 2 passes):  celu(w)+a == max(w+a, min(a*e^{10 w}, a))
        ACT:  u' = Exp(10*psum + (ln a - 10 a)) -> bf16    [psum holds w+a]
        DVE:  h = (u' MIN a) MAX psum  (one scalar_tensor_tensor)
        +a shift folded into next layer's bias: b_eff = b - a*sum_fin(Wq) + a.
    L2: f32r (exact) weights x f32r h1; L3: fp16 W3 x fp16 h2, with the
        8 models' 96-wide outputs packed into 6 full 128-partition PSUM
        banks (pieces at 32-aligned tile positions) so drains touch 6
        banks instead of 8; f32r bias matmuls (one per bank).
    readout: 6 accumulating W4 (f32r, bank-concatenated) matmuls ->
        e[1, 512] PSUM row, ACT Identity accum -> acc column per unit.
  PSUM: 2-bank slots rotating 4-deep; emission interleaves unit u+1's L1
  slots through unit u's L2/L3/readout so no engine waits on drains.
  Host: subtract a*sum(W4) shift terms, add b4 terms, evaluate the
  4*212 leftover atoms exactly in f64.
"""
import math
import numpy as np
import ml_dtypes

BF16 = ml_dtypes.bfloat16
E4M3 = ml_dtypes.float8_e4m3fn if hasattr(ml_dtypes, 'float8_e4m3fn') else ml_dtypes.float8_e4m3

S = 4
M = 8
F0, F1, F2, F3 = 384, 160, 128, 96
ALPHA = 0.1
N_CORES = 8
T = 512
C_MAX = 1536
LNAC = math.log(ALPHA) - 10.0 * ALPHA

# h1 tile block layout: drain-order blocks; chunk -> block
#   slot1 = chunks [8,9,0,1] -> blocks 0..3
#   slot2 = chunks [2,3,4,5] -> blocks 4..7
#   slot3 = chunks [6,7]     -> blocks 8..9
CHUNK_ORDER = [8, 9, 0, 1, 2, 3, 4, 5, 6, 7]
BLK = {c: i for i, c in enumerate(CHUNK_ORDER)}

# L3 output packing: 8 models x 96 feats -> 6 full 128-partition banks.
# Pieces per 4-model group g (base model 4g, base bank 3g):
#   (model, f0, f1, bank, partition offset)
L3_PIECES = []
for _g in (0, 1):
    _M, _B = 4 * _g, 3 * _g
    L3_PIECES += [
        (_M + 0, 0, 96, _B + 0, 0),
        (_M + 1, 0, 32, _B + 0, 96),
        (_M + 1, 32, 96, _B + 1, 0),
        (_M + 2, 0, 64, _B + 1, 64),
        (_M + 2, 64, 96, _B + 2, 0),
        (_M + 3, 0, 32, _B + 2, 32),
        (_M + 3, 32, 64, _B + 2, 64),
        (_M + 3, 64, 96, _B + 2, 96),
    ]
# piece -> column offset in the packed w3 tile (piece-major)
L3_PCOL = []
_off = 0
for (_m, _f0, _f1, _b, _o) in L3_PIECES:
    L3_PCOL.append(_off)
    _off += _f1 - _f0

_BUILD_CACHE = {}


def build_kernel(C=C_MAX):
    if C in _BUILD_CACHE:
        return _BUILD_CACHE[C]
    NT = C // T
    N_UNITS = S * NT

    import concourse.bacc as bacc
    import concourse.tile as tile
    import concourse.mybir as mybir

    F32 = mybir.dt.float32
    F32R = mybir.dt.float32r
    DBF = mybir.dt.bfloat16
    DF16 = mybir.dt.float16
    DE4 = mybir.dt.float8e4
    MIN, MAX, ADD = mybir.AluOpType.min, mybir.AluOpType.max, mybir.AluOpType.add
    EXP = mybir.ActivationFunctionType.Exp
    RELU = mybir.ActivationFunctionType.Relu
    DR = mybir.MatmulPerfMode.DoubleRow
    BANK = 512

    nc = bacc.Bacc("TRN2", target_bir_lowering=False, debug=False)

    xa_d = nc.dram_tensor("xa", [S, NT, 128, 2 * T], DE4, kind="ExternalInput").ap()
    xb_d = nc.dram_tensor("xb", [S, NT, 65, 2 * T], DE4, kind="ExternalInput").ap()
    w1a_d = nc.dram_tensor("w1a", [S, 128, 10 * 256], DE4, kind="ExternalInput").ap()
    w1b_d = nc.dram_tensor("w1b", [S, 65, 10 * 256], DE4, kind="ExternalInput").ap()
    w2m_d = nc.dram_tensor("w2m", [S, 128, M * 128], F32R, kind="ExternalInput").ap()
    w2r_d = nc.dram_tensor("w2r", [S, 128, 2 * 128], F32R, kind="ExternalInput").ap()
    w3_d = nc.dram_tensor("w3", [S, 128, M * 96], DF16, kind="ExternalInput").ap()
    w4_d = nc.dram_tensor("w4", [S, 128, 6], F32R, kind="ExternalInput").ap()
    bl2_d = nc.dram_tensor("bl2", [S, 128, 256], F32R, kind="ExternalInput").ap()
    bl3_d = nc.dram_tensor("bl3", [S, 128, 256], F32R, kind="ExternalInput").ap()
    ones_d = nc.dram_tensor("ones", [128, T], F32R, kind="ExternalInput").ap()
    acc_d = nc.dram_tensor("acc", [1, N_UNITS], F32, kind="ExternalOutput").ap()

    with tile.TileContext(nc) as tc:
        with tc.tile_pool(name="wpool", bufs=1) as wpool, \
             tc.tile_pool(name="xpool", bufs=2) as xpool, \
             tc.tile_pool(name="hpool", bufs=2) as hpool, \
             tc.tile_pool(name="h2pool", bufs=1) as h2pool, \
             tc.tile_pool(name="upool", bufs=3) as upool, \
             tc.tile_pool(name="apool", bufs=1) as apool, \
             tc.tile_pool(name="ps", bufs=4, space="PSUM") as psp:

            w1a_t, w1b_t, w2m_t, w2r_t, w3_t, w4_t, bl2_t, bl3_t = ({} for _ in range(8))
            for s in range(S):
                w1a_t[s] = wpool.tile([128, 10 * 256], DE4, tag=f"w1a{s}", name=f"w1a{s}")
                w1b_t[s] = wpool.tile([65, 10 * 256], DE4, tag=f"w1b{s}", name=f"w1b{s}")
                w2m_t[s] = wpool.tile([128, M * 128], F32R, tag=f"w2m{s}", name=f"w2m{s}")
                w2r_t[s] = wpool.tile([128, 2 * 128], F32R, tag=f"w2r{s}", name=f"w2r{s}")
                w3_t[s] = wpool.tile([128, M * 96], DF16, tag=f"w3{s}", name=f"w3{s}")
                w4_t[s] = wpool.tile([128, 6], F32R, tag=f"w4{s}", name=f"w4{s}")
                bl2_t[s] = wpool.tile([128, 256], F32R, tag=f"bl2{s}", name=f"bl2{s}")
                bl3_t[s] = wpool.tile([128, 256], F32R, tag=f"bl3{s}", name=f"bl3{s}")

            def dma_l1_weights(s, split=False):
                nc.sync.dma_start(w1a_t[s][:, 0:3 * 256], w1a_d[s][:, 0:3 * 256])
                nc.sync.dma_start(w1b_t[s][:, 0:3 * 256], w1b_d[s][:, 0:3 * 256])
                if not split:
                    nc.sync.dma_start(w1a_t[s][:, 3 * 256:], w1a_d[s][:, 3 * 256:])
                    nc.sync.dma_start(w1b_t[s][:, 3 * 256:], w1b_d[s][:, 3 * 256:])

            def dma_l1_weights_rest(s):
                nc.sync.dma_start(w1a_t[s][:, 3 * 256:], w1a_d[s][:, 3 * 256:])
                nc.sync.dma_start(w1b_t[s][:, 3 * 256:], w1b_d[s][:, 3 * 256:])

            def dma_rest_weights(s):
                for tt, dd in ((w2m_t[s], w2m_d[s]), (w2r_t[s], w2r_d[s]),
                               (w3_t[s], w3_d[s]), (w4_t[s], w4_d[s]),
                               (bl2_t[s], bl2_d[s]), (bl3_t[s], bl3_d[s])):
                    nc.sync.dma_start(tt[:], dd)

            ones_t = wpool.tile([128, T], F32R, tag="ones", name="ones")
            nc.sync.dma_start(ones_t[:], ones_d)
            lnac_t = wpool.tile([128, 1], F32, tag="lnac", name="lnac")
            nc.vector.memset(lnac_t[:], LNAC)
            # warm the ACT Exp table during the initial DMA window
            warm_t = wpool.tile([128, 1], F32, tag="warm", name="warm")
            nc.scalar.activation(warm_t[:], lnac_t[:],
                                 mybir.ActivationFunctionType.Exp,
                                 bias=0.0, scale=1.0)
            acc_t = apool.tile([1, N_UNITS], F32, tag="acc", name="acc")
            junk_t = apool.tile([1, T], F32, tag="junk", name="junk")

            def drain(ps_t, nrow, gsz, h_view):
                ps_v = ps_t[0:nrow, :].rearrange("p (g q) -> p g q", q=BANK)[:, 0:gsz, 0:T]
                u = upool.tile([128, 2 * T], DBF, tag="u", name="u")
                u_v = u[0:nrow, 0:gsz * T].rearrange("p (g q) -> p g q", q=T)
                nc.scalar.activation(u_v, ps_v, EXP, bias=lnac_t[0:nrow, 0:1], scale=10.0)
                nc.vector.scalar_tensor_tensor(h_view, u_v, ALPHA, ps_v, op0=MIN, op1=MAX)

            def hview(h_t, b0, g, nrow=128):
                return h_t[0:nrow, b0 * T:(b0 + g) * T].rearrange("p (g q) -> p g q", q=T)

            def l1_slot_fns(s, xa_v, xb_v, h1):
                def mk(slot_i, chunks):
                    def fn():
                        ps_t = psp.tile([128, 2 * BANK], F32, tag="psg", name=f"l1s{slot_i}")
                        for g, c in enumerate(chunks):
                            k = BLK[c]  # host packs chunk CHUNK_ORDER[k] at col block k
                            nc.tensor.matmul(
                                ps_t[:, g * BANK:g * BANK + T],
                                w1a_t[s][:, k * 256:(k + 1) * 256].rearrange(
                                    "p (i m) -> p i m", i=2),
                                xa_v, start=True, stop=False, perf_mode=DR)
                            nc.tensor.matmul(
                                ps_t[:, g * BANK:g * BANK + T],
                                w1b_t[s][:, k * 256:(k + 1) * 256].rearrange(
                                    "p (i m) -> p i m", i=2),
                                xb_v, start=False, stop=True, perf_mode=DR)
                        drain(ps_t, 128, len(chunks),
                              hview(h1, BLK[chunks[0]], len(chunks)))
                    return fn
                return [mk(i, ch) for i, ch in
                        enumerate(([8, 9], [0, 1], [2, 3], [4, 5], [6, 7]))]

            def l2_slot_fns(s, h1, h2):
                def mk(m0):
                  def fn():
                    ps_t = psp.tile([128, 2 * BANK], F32, tag="psg", name=f"l2m{m0}")
                    for g in range(2):
                        m = m0 + g
                        reg, j = m // 4, m % 4
                        bm = BLK[m]
                        sl = slice(g * BANK, g * BANK + T)
                        nc.tensor.matmul(
                            ps_t[:, sl],
                            w2m_t[s][:, m * 128:(m + 1) * 128],
                            h1[:, bm * T:(bm + 1) * T],
                            start=True, stop=False)
                        nc.tensor.matmul(
                            ps_t[:, sl],
                            w2r_t[s][32 * j:32 * (j + 1), reg * 128:(reg + 1) * 128],
                            h1[32 * j:32 * (j + 1), BLK[8 + reg] * T:(BLK[8 + reg] + 1) * T],
                            start=False, stop=False, tile_position=(32 * j, 0))
                        nc.tensor.matmul(
                            ps_t[0:128, sl],
                            bl2_t[s][32 * j:32 * j + 1, reg * 128:reg * 128 + 128],
                            ones_t[32 * j:32 * j + 1, 0:T],
                            start=False, stop=True, tile_position=(32 * j, 0))
                    drain(ps_t, 128, 2, hview(h2, m0, 2))
                  return fn
                return [mk(m0) for m0 in (0, 2, 4, 6)]

            def l3_slot_fns(s, h2, h3):
                # packed L3: 6 full banks, pieces per L3_PIECES
                def mk(b0):
                  def fn():
                    ps_t = psp.tile([128, 2 * BANK], F32, tag="psg", name=f"l3b{b0}")
                    for g in range(2):
                        bank = b0 + g
                        sl = slice(g * BANK, g * BANK + T)
                        for pi, (m, f0, f1, bk, o) in enumerate(L3_PIECES):
                            if bk != bank:
                                continue
                            w = f1 - f0
                            # pieces cover disjoint partition ranges: each
                            # must open its own accumulation region
                            nc.tensor.matmul(
                                ps_t[o:o + w, sl],
                                w3_t[s][:, L3_PCOL[pi]:L3_PCOL[pi] + w],
                                h2[:, m * T:(m + 1) * T],
                                start=True, stop=False,
                                tile_position=(0, o))
                        j, blk = bank % 4, bank // 4
                        nc.tensor.matmul(
                            ps_t[0:128, sl],
                            bl3_t[s][32 * j:32 * j + 1, blk * 128:blk * 128 + 128],
                            ones_t[32 * j:32 * j + 1, 0:T],
                            start=False, stop=True, tile_position=(32 * j, 0))
                    drain(ps_t, 128, 2, hview(h3, b0, 2))
                  return fn
                return [mk(b0) for b0 in (0, 2, 4)]

            def readout_fn(s, unit, h3):
              def fn():
                ps_t = psp.tile([128, 2 * BANK], F32, tag="psg", name="eps")
                for k in range(6):
                    nc.tensor.matmul(
                        ps_t[0:1, 0:T],
                        w4_t[s][:, k:k + 1],
                        h3[:, k * T:(k + 1) * T],
                        start=(k == 0), stop=(k == 5))
                nc.scalar.activation(
                    junk_t[:], ps_t[0:1, 0:T],
                    mybir.ActivationFunctionType.Identity,
                    bias=0.0, scale=1.0,
                    accum_out=acc_t[0:1, unit:unit + 1])
              return fn

            units = [(s, t) for s in range(S) for t in range(NT)]
            pending = None
            dma_l1_weights(0, split=True)
            for unit, (s, t) in enumerate(units):
                xa_t = xpool.tile([128, 2 * T], DE4, tag="xa", name="xa")
                xb_t = xpool.tile([65, 2 * T], DE4, tag="xb", name="xb")
                nc.sync.dma_start(xa_t[:], xa_d[s, t])
                nc.sync.dma_start(xb_t[:], xb_d[s, t])
                if unit == 0:
                    dma_l1_weights_rest(0)
                if t == 0:
                    dma_rest_weights(s)
                if t == NT - 1 and s + 1 < S:
                    dma_l1_weights(s + 1)
                h1 = hpool.tile([128, 10 * T], F32R, tag="h1", name="h1")
                h2 = h2pool.tile([128, 8 * T], DF16, tag="h2", name="h2")
                h3 = h2pool.tile([128, 6 * T], F32R, tag="h3", name="h3")
                l1f = l1_slot_fns(s, xa_t[:].rearrange("p (i q) -> p i q", i=2),
                                  xb_t[:].rearrange("p (i q) -> p i q", i=2), h1)
                s2f = []
                if pending is not None:
                    us_, uu_, h1_, h2_, h3_ = pending
                    s2f = (l2_slot_fns(us_, h1_, h2_) + l3_slot_fns(us_, h2_, h3_)
                           + [readout_fn(us_, uu_, h3_)])
                # interleave ~1 L1 slot per 2 stage-2 slots, L1 first
                if s2f:
                    order = [l1f[0], s2f[0], s2f[1], l1f[1], s2f[2], s2f[3],
                             l1f[2], s2f[4], s2f[5], s2f[6],
                             l1f[3], l1f[4], s2f[7]]
                else:
                    order = l1f
                for fn in order:
                    fn()
                pending = (s, unit, h1, h2, h3)
            us_, uu_, h1_, h2_, h3_ = pending
            for fn in (l2_slot_fns(us_, h1_, h2_) + l3_slot_fns(us_, h2_, h3_)
                       + [readout_fn(us_, uu_, h3_)]):
                fn()

            nc.sync.dma_start(acc_d, acc_t[:])

    nc.compile()
    _BUILD_CACHE[C] = nc
    return nc


# ----------------------------------------------------------------------------
# host-side packing
# ----------------------------------------------------------------------------
def _celu64(x):
    return np.where(x > 0, x, ALPHA * np.expm1(np.minimum(x, 0) / ALPHA))


def _q(x, dt):
    return np.asarray(x).astype(dt).astype(np.float64)


def prep_inputs(species, aev, W1, b1, W2, b2, W3, b3, W4, b4):
    sp = np.asarray(species).reshape(-1)
    n_atoms = sp.shape[0]
    aev0 = np.asarray(aev, dtype=np.float32).reshape(n_atoms, F0)
    W1, b1, W2, b2, W3, b3, W4, b4 = [np.asarray(a, np.float64) for a in
                                      (W1, b1, W2, b2, W3, b3, W4, b4)]

    order = np.argsort(sp, kind="stable")
    cnt = np.bincount(sp.astype(np.int64), minlength=S)
    starts = np.concatenate([[0], np.cumsum(cnt)])
    # device capacity: largest tile-multiple that every species fills exactly
    # (no padding on device); overflow atoms are evaluated on the host in f64
    C = min(C_MAX, (int(cnt.min()) // (N_CORES * T)) * T)
    assert C >= T, "species too unbalanced for device path"
    NT = C // T
    N_UNITS = S * NT
    dev_cnt = np.minimum(cnt, N_CORES * C)

    aev8 = aev0.astype(E4M3).astype(np.float32)

    xas = [np.zeros((S, NT, 128, 2 * T), dtype=E4M3) for _ in range(N_CORES)]
    xbs = [np.zeros((S, NT, 65, 2 * T), dtype=E4M3) for _ in range(N_CORES)]
    for s in range(S):
        idx = order[starts[s]:starts[s] + dev_cnt[s]]
        blk = aev8[idx]
        for c in range(N_CORES):
            seg = blk[c * C:(c + 1) * C]
            xf = np.concatenate([seg.T, np.ones((2, C), np.float32)], axis=0)
            for t in range(NT):
                col = xf[:, t * T:(t + 1) * T]
                xas[c][s, t] = col[0:256].reshape(128, 2 * T).astype(E4M3)
                xbs[c][s, t] = col[256:386].reshape(65, 2 * T).astype(E4M3)

    # W1 DR-packed with bias + bias-residual rows (fp8)
    w1a = np.zeros((S, 128, 10 * 256), dtype=E4M3)
    w1b = np.zeros((S, 65, 10 * 256), dtype=E4M3)
    for s in range(S):
        cols = np.zeros((F0, 1280), np.float64)
        brow = np.zeros(1280, np.float64)
        for c in range(8):
            cols[:, 128 * c:128 * (c + 1)] = W1[s, c, :, 0:128]
            brow[128 * c:128 * (c + 1)] = b1[s, c, 0, 0:128] + ALPHA
        for r in range(2):
            for j in range(4):
                c0 = 1024 + 128 * r + 32 * j
                cols[:, c0:c0 + 32] = W1[s, 4 * r + j, :, 128:160]
                brow[c0:c0 + 32] = b1[s, 4 * r + j, 0, 128:160] + ALPHA
        b_hi = _q(brow, E4M3)
        b_lo = brow - b_hi
        full = np.concatenate([cols, b_hi[None, :], b_lo[None, :]], axis=0)
        fq = full.astype(np.float32).astype(E4M3)
        for k, c in enumerate(CHUNK_ORDER):
            blkc = fq[:, 128 * c:128 * (c + 1)].astype(np.float32)
            w1a[s][:, k * 256:(k + 1) * 256] = blkc[0:256].reshape(128, 256).astype(E4M3)
            w1b[s][:, k * 256:(k + 1) * 256] = blkc[256:386].reshape(65, 256).astype(E4M3)

    w2m = np.zeros((S, 128, M * 128), np.float32)
    w2r = np.zeros((S, 128, 2 * 128), np.float32)
    bl2 = np.zeros((S, 128, 256), np.float32)
    for s in range(S):
        for m in range(M):
            reg, j = m // 4, m % 4
            w2m[s][:, m * 128:(m + 1) * 128] = W2[s, m, 0:128]
            w2r[s][32 * j:32 * (j + 1), reg * 128:(reg + 1) * 128] = W2[s, m, 128:160]
            beff = b2[s, m, 0, :] - ALPHA * W2[s, m].sum(axis=0) + ALPHA
            bl2[s][32 * j, reg * 128:reg * 128 + 128] = beff.astype(np.float32)

    w3p = np.zeros((S, 128, M * 96), dtype=np.float16)
    bl3 = np.zeros((S, 128, 256), np.float32)
    w4p = np.zeros((S, 128, 6), np.float32)
    for s in range(S):
        W3q = _q(W3[s].astype(np.float32), np.float16)     # device-quantized W3
        beff3 = np.stack([b3[s, m, 0, :] - ALPHA * W3q[m].sum(axis=0) + ALPHA
                          for m in range(M)])          # [M, 96]
        for pi, (m, f0, f1, bank, o) in enumerate(L3_PIECES):
            w = f1 - f0
            w3p[s][:, L3_PCOL[pi]:L3_PCOL[pi] + w] = W3q[m, :, f0:f1].astype(np.float16)
            j, blk = bank % 4, bank // 4
            bl3[s][32 * j, blk * 128 + o:blk * 128 + o + w] = \
                beff3[m, f0:f1].astype(np.float32)
            w4p[s][o:o + w, bank] = W4[s, m, f0:f1, 0].astype(np.float32)

    common = {"w1a": w1a, "w1b": w1b, "w2m": w2m, "w2r": w2r, "w3": w3p,
              "w4": w4p, "bl2": bl2, "bl3": bl3,
              "ones": np.ones((128, T), np.float32)}
    in_maps = [dict(common, xa=xas[c], xb=xbs[c]) for c in range(N_CORES)]

    w4sum = np.array([_q(W4[s, :, :, 0].astype(np.float32), np.float64).sum()
                      for s in range(S)])
    b4sum = b4[:, :, 0, 0].sum(axis=1)

    leftover = 0.0
    for s in range(S):
        n_left = int(cnt[s] - dev_cnt[s])
        if n_left <= 0:
            continue
        idx = order[starts[s] + dev_cnt[s]:starts[s + 1]]
        x = aev0[idx].astype(np.float64)
        for m in range(M):
            h = _celu64(x @ W1[s, m] + b1[s, m, 0])
            h = _celu64(h @ W2[s, m] + b2[s, m, 0])
            h = _celu64(h @ W3[s, m] + b3[s, m, 0])
            leftover += float((h @ W4[s, m, :, 0]).sum()) + n_left * float(b4[s, m, 0, 0])

    def finish(results):
        tot = 0.0
        for res in results:
            a = res["acc"].astype(np.float64).reshape(N_UNITS)
            for s in range(S):
                for t in range(NT):
                    tot += a[s * NT + t] - T * ALPHA * w4sum[s]
        for s in range(S):
            tot += dev_cnt[s] * b4sum[s]
        tot += leftover
        return np.array([tot / M], dtype=np.float32)

    return C, in_maps, finish


def _ensure_axon_platform():
    try:
        import jax
        devs = jax.devices()
        if len(devs) >= N_CORES and devs[0].platform != "cpu":
            return
        jax.config.update("jax_platforms", "axon")
    except Exception:
        pass


def kernel(**inputs):
    from concourse.bass_utils import run_bass_kernel_spmd
    _ensure_axon_platform()
    C, in_maps, finish = prep_inputs(**inputs)
    nc = build_kernel(C)
    res = run_bass_kernel_spmd(nc, in_maps, list(range(N_CORES)))
    return finish(res.results)


# revision 7
# speedup vs baseline: 1.0623x; 1.0050x over previous
"""Trainium2 Bass kernel for nn_BmmEnsemble (species-routed CELU-MLP ensemble).

Strategy (data-parallel over atoms, 8 NeuronCores):
  host: stable-sort atoms by species, shard species blocks across cores
        (C=1536 atoms/species/core, no padding), fp8-quantize aev,
        pack DoubleRow operands, pre-fold biases.
  device per tile-unit (species s, 512-atom tile t):
    L1: z+b+a in PSUM via fp8e4m3 DoubleRow matmuls (2 per 128-out chunk,
        K=386 = 384 aev rows + fp8 bias row + fp8 bias-residual row).
    drain (exact CELU, 2 passes):  celu(w)+a == max(w+a, min(a*e^{10 w}, a))
        ACT:  u' = Exp(10*psum + (ln a - 10 a)) -> bf16    [psum holds w+a]
        DVE:  h = (u' MIN a) MAX psum  (one scalar_tensor_tensor)
        +a shift folded into next layer's bias: b_eff = b - a*sum_fin(Wq) + a.
    L2: f32r (exact) weights x f32r h1; L3: fp16 W3 x fp16 h2, with the
        8 models' 96-wide outputs packed into 6 full 128-partition PSUM
        banks (pieces at 32-aligned tile positions) so drains touch 6
        banks instead of 8; f32r bias matmuls (one per bank).
    readout: 6 accumulating W4 (f32r, bank-concatenated) matmuls ->
        e[1, 512] PSUM row, ACT Identity accum -> acc column per unit.
  PSUM: 2-bank slots rotating 4-deep; emission interleaves unit u+1's L1
  slots through unit u's L2/L3/readout so no engine waits on drains.
  Host: subtract a*sum(W4) shift terms, add b4 terms, evaluate the
  4*212 leftover atoms exactly in f64.
"""
import math
import numpy as np
import ml_dtypes

BF16 = ml_dtypes.bfloat16
E4M3 = ml_dtypes.float8_e4m3fn if hasattr(ml_dtypes, 'float8_e4m3fn') else ml_dtypes.float8_e4m3

S = 4
M = 8
F0, F1, F2, F3 = 384, 160, 128, 96
ALPHA = 0.1
N_CORES = 8
T = 512
C_MAX = 1536
LNAC = math.log(ALPHA) - 10.0 * ALPHA

# h1 tile block layout: drain-order blocks; chunk -> block
#   slot1 = chunks [8,9,0,1] -> blocks 0..3
#   slot2 = chunks [2,3,4,5] -> blocks 4..7
#   slot3 = chunks [6,7]     -> blocks 8..9
CHUNK_ORDER = [8, 9, 0, 1, 2, 3, 4, 5, 6, 7]
BLK = {c: i for i, c in enumerate(CHUNK_ORDER)}

# L3 output packing: 8 models x 96 feats -> 6 full 128-partition banks.
# Pieces per 4-model group g (base model 4g, base bank 3g):
#   (model, f0, f1, bank, partition offset)
L3_PIECES = []
for _g in (0, 1):
    _M, _B = 4 * _g, 3 * _g
    L3_PIECES += [
        (_M + 0, 0, 96, _B + 0, 0),
        (_M + 1, 0, 32, _B + 0, 96),
        (_M + 1, 32, 96, _B + 1, 0),
        (_M + 2, 0, 64, _B + 1, 64),
        (_M + 2, 64, 96, _B + 2, 0),
        (_M + 3, 0, 32, _B + 2, 32),
        (_M + 3, 32, 64, _B + 2, 64),
        (_M + 3, 64, 96, _B + 2, 96),
    ]
# piece -> column offset in the packed w3 tile (piece-major)
L3_PCOL = []
_off = 0
for (_m, _f0, _f1, _b, _o) in L3_PIECES:
    L3_PCOL.append(_off)
    _off += _f1 - _f0

_BUILD_CACHE = {}


def build_kernel(C=C_MAX):
    if C in _BUILD_CACHE:
        return _BUILD_CACHE[C]
    NT = C // T
    N_UNITS = S * NT

    import concourse.bacc as bacc
    import concourse.tile as tile
    import concourse.mybir as mybir

    F32 = mybir.dt.float32
    F32R = mybir.dt.float32r
    DBF = mybir.dt.bfloat16
    DF16 = mybir.dt.float16
    DE4 = mybir.dt.float8e4
    MIN, MAX, ADD = mybir.AluOpType.min, mybir.AluOpType.max, mybir.AluOpType.add
    EXP = mybir.ActivationFunctionType.Exp
    RELU = mybir.ActivationFunctionType.Relu
    DR = mybir.MatmulPerfMode.DoubleRow
    BANK = 512

    nc = bacc.Bacc("TRN2", target_bir_lowering=False, debug=False)

    xa_d = nc.dram_tensor("xa", [S, NT, 128, 2 * T], DE4, kind="ExternalInput").ap()
    xb_d = nc.dram_tensor("xb", [S, NT, 65, 2 * T], DE4, kind="ExternalInput").ap()
    w1a_d = nc.dram_tensor("w1a", [S, 128, 10 * 256], DE4, kind="ExternalInput").ap()
    w1b_d = nc.dram_tensor("w1b", [S, 65, 10 * 256], DE4, kind="ExternalInput").ap()
    w2m_d = nc.dram_tensor("w2m", [S, 128, M * 128], F32R, kind="ExternalInput").ap()
    w2r_d = nc.dram_tensor("w2r", [S, 128, 2 * 128], F32R, kind="ExternalInput").ap()
    w3_d = nc.dram_tensor("w3", [S, 128, M * 96], DF16, kind="ExternalInput").ap()
    w4_d = nc.dram_tensor("w4", [S, 128, 6], F32R, kind="ExternalInput").ap()
    bl2_d = nc.dram_tensor("bl2", [S, 128, 256], F32R, kind="ExternalInput").ap()
    bl3_d = nc.dram_tensor("bl3", [S, 128, 256], F32R, kind="ExternalInput").ap()
    ones_d = nc.dram_tensor("ones", [128, T], F32R, kind="ExternalInput").ap()
    acc_d = nc.dram_tensor("acc", [1, N_UNITS], F32, kind="ExternalOutput").ap()

    with tile.TileContext(nc) as tc:
        with tc.tile_pool(name="wpool", bufs=1) as wpool, \
             tc.tile_pool(name="xpool", bufs=2) as xpool, \
             tc.tile_pool(name="hpool", bufs=2) as hpool, \
             tc.tile_pool(name="h2pool", bufs=1) as h2pool, \
             tc.tile_pool(name="upool", bufs=3) as upool, \
             tc.tile_pool(name="apool", bufs=1) as apool, \
             tc.tile_pool(name="ps", bufs=4, space="PSUM") as psp:

            w1a_t, w1b_t, w2m_t, w2r_t, w3_t, w4_t, bl2_t, bl3_t = ({} for _ in range(8))
            for s in range(S):
                w1a_t[s] = wpool.tile([128, 10 * 256], DE4, tag=f"w1a{s}", name=f"w1a{s}")
                w1b_t[s] = wpool.tile([65, 10 * 256], DE4, tag=f"w1b{s}", name=f"w1b{s}")
                w2m_t[s] = wpool.tile([128, M * 128], F32R, tag=f"w2m{s}", name=f"w2m{s}")
                w2r_t[s] = wpool.tile([128, 2 * 128], F32R, tag=f"w2r{s}", name=f"w2r{s}")
                w3_t[s] = wpool.tile([128, M * 96], DF16, tag=f"w3{s}", name=f"w3{s}")
                w4_t[s] = wpool.tile([128, 6], F32R, tag=f"w4{s}", name=f"w4{s}")
                bl2_t[s] = wpool.tile([128, 256], F32R, tag=f"bl2{s}", name=f"bl2{s}")
                bl3_t[s] = wpool.tile([128, 256], F32R, tag=f"bl3{s}", name=f"bl3{s}")

            def dma_l1_weights(s, split=False):
                nc.sync.dma_start(w1a_t[s][:, 0:4 * 256], w1a_d[s][:, 0:4 * 256])
                nc.sync.dma_start(w1b_t[s][:, 0:4 * 256], w1b_d[s][:, 0:4 * 256])
                if not split:
                    nc.sync.dma_start(w1a_t[s][:, 4 * 256:], w1a_d[s][:, 4 * 256:])
                    nc.sync.dma_start(w1b_t[s][:, 4 * 256:], w1b_d[s][:, 4 * 256:])

            def dma_l1_weights_rest(s):
                nc.sync.dma_start(w1a_t[s][:, 4 * 256:], w1a_d[s][:, 4 * 256:])
                nc.sync.dma_start(w1b_t[s][:, 4 * 256:], w1b_d[s][:, 4 * 256:])

            def dma_rest_weights(s):
                for tt, dd in ((w2m_t[s], w2m_d[s]), (w2r_t[s], w2r_d[s]),
                               (w3_t[s], w3_d[s]), (w4_t[s], w4_d[s]),
                               (bl2_t[s], bl2_d[s]), (bl3_t[s], bl3_d[s])):
                    nc.sync.dma_start(tt[:], dd)

            ones_t = wpool.tile([128, T], F32R, tag="ones", name="ones")
            lnac_t = wpool.tile([128, 1], F32, tag="lnac", name="lnac")
            nc.vector.memset(lnac_t[:], LNAC)
            # warm the ACT Exp table during the initial DMA window
            warm_t = wpool.tile([128, 1], F32, tag="warm", name="warm")
            nc.scalar.activation(warm_t[:], lnac_t[:],
                                 mybir.ActivationFunctionType.Exp,
                                 bias=0.0, scale=1.0)
            acc_t = apool.tile([1, N_UNITS], F32, tag="acc", name="acc")
            junk_t = apool.tile([1, T], F32, tag="junk", name="junk")

            def drain(ps_t, nrow, gsz, h_view):
                ps_v = ps_t[0:nrow, :].rearrange("p (g q) -> p g q", q=BANK)[:, 0:gsz, 0:T]
                u = upool.tile([128, 2 * T], DBF, tag="u", name="u")
                u_v = u[0:nrow, 0:gsz * T].rearrange("p (g q) -> p g q", q=T)
                nc.scalar.activation(u_v, ps_v, EXP, bias=lnac_t[0:nrow, 0:1], scale=10.0)
                nc.vector.scalar_tensor_tensor(h_view, u_v, ALPHA, ps_v, op0=MIN, op1=MAX)

            def hview(h_t, b0, g, nrow=128):
                return h_t[0:nrow, b0 * T:(b0 + g) * T].rearrange("p (g q) -> p g q", q=T)

            def l1_slot_fns(s, xa_v, xb_v, h1):
                def mk(slot_i, chunks):
                    def fn():
                        ps_t = psp.tile([128, 2 * BANK], F32, tag="psg", name=f"l1s{slot_i}")
                        for g, c in enumerate(chunks):
                            k = BLK[c]  # host packs chunk CHUNK_ORDER[k] at col block k
                            nc.tensor.matmul(
                                ps_t[:, g * BANK:g * BANK + T],
                                w1a_t[s][:, k * 256:(k + 1) * 256].rearrange(
                                    "p (i m) -> p i m", i=2),
                                xa_v, start=True, stop=False, perf_mode=DR)
                            nc.tensor.matmul(
                                ps_t[:, g * BANK:g * BANK + T],
                                w1b_t[s][:, k * 256:(k + 1) * 256].rearrange(
                                    "p (i m) -> p i m", i=2),
                                xb_v, start=False, stop=True, perf_mode=DR)
                        drain(ps_t, 128, len(chunks),
                              hview(h1, BLK[chunks[0]], len(chunks)))
                    return fn
                return [mk(i, ch) for i, ch in
                        enumerate(([8, 9], [0, 1], [2, 3], [4, 5], [6, 7]))]

            def l2_slot_fns(s, h1, h2):
                def mk(m0):
                  def fn():
                    ps_t = psp.tile([128, 2 * BANK], F32, tag="psg", name=f"l2m{m0}")
                    for g in range(2):
                        m = m0 + g
                        reg, j = m // 4, m % 4
                        bm = BLK[m]
                        sl = slice(g * BANK, g * BANK + T)
                        nc.tensor.matmul(
                            ps_t[:, sl],
                            w2m_t[s][:, m * 128:(m + 1) * 128],
                            h1[:, bm * T:(bm + 1) * T],
                            start=True, stop=False)
                        nc.tensor.matmul(
                            ps_t[:, sl],
                            w2r_t[s][32 * j:32 * (j + 1), reg * 128:(reg + 1) * 128],
                            h1[32 * j:32 * (j + 1), BLK[8 + reg] * T:(BLK[8 + reg] + 1) * T],
                            start=False, stop=False, tile_position=(32 * j, 0))
                        nc.tensor.matmul(
                            ps_t[0:128, sl],
                            bl2_t[s][32 * j:32 * j + 1, reg * 128:reg * 128 + 128],
                            ones_t[32 * j:32 * j + 1, 0:T],
                            start=False, stop=True, tile_position=(32 * j, 0))
                    drain(ps_t, 128, 2, hview(h2, m0, 2))
                  return fn
                return [mk(m0) for m0 in (0, 2, 4, 6)]

            def l3_slot_fns(s, h2, h3):
                # packed L3: 6 full banks, pieces per L3_PIECES
                def mk(b0):
                  def fn():
                    ps_t = psp.tile([128, 2 * BANK], F32, tag="psg", name=f"l3b{b0}")
                    for g in range(2):
                        bank = b0 + g
                        sl = slice(g * BANK, g * BANK + T)
                        for pi, (m, f0, f1, bk, o) in enumerate(L3_PIECES):
                            if bk != bank:
                                continue
                            w = f1 - f0
                            # pieces cover disjoint partition ranges: each
                            # must open its own accumulation region
                            nc.tensor.matmul(
                                ps_t[o:o + w, sl],
                                w3_t[s][:, L3_PCOL[pi]:L3_PCOL[pi] + w],
                                h2[:, m * T:(m + 1) * T],
                                start=True, stop=False,
                                tile_position=(0, o))
                        j, blk = bank % 4, bank // 4
                        nc.tensor.matmul(
                            ps_t[0:128, sl],
                            bl3_t[s][32 * j:32 * j + 1, blk * 128:blk * 128 + 128],
                            ones_t[32 * j:32 * j + 1, 0:T],
                            start=False, stop=True, tile_position=(32 * j, 0))
                    drain(ps_t, 128, 2, hview(h3, b0, 2))
                  return fn
                return [mk(b0) for b0 in (0, 2, 4)]

            def readout_fn(s, unit, h3):
              def fn():
                ps_t = psp.tile([128, 2 * BANK], F32, tag="psg", name="eps")
                for k in range(6):
                    nc.tensor.matmul(
                        ps_t[0:1, 0:T],
                        w4_t[s][:, k:k + 1],
                        h3[:, k * T:(k + 1) * T],
                        start=(k == 0), stop=(k == 5))
                nc.scalar.activation(
                    junk_t[:], ps_t[0:1, 0:T],
                    mybir.ActivationFunctionType.Identity,
                    bias=0.0, scale=1.0,
                    accum_out=acc_t[0:1, unit:unit + 1])
              return fn

            units = [(s, t) for s in range(S) for t in range(NT)]
            pending = None
            dma_l1_weights(0, split=True)
            for unit, (s, t) in enumerate(units):
                xa_t = xpool.tile([128, 2 * T], DE4, tag="xa", name="xa")
                xb_t = xpool.tile([65, 2 * T], DE4, tag="xb", name="xb")
                nc.sync.dma_start(xa_t[:], xa_d[s, t])
                nc.sync.dma_start(xb_t[:], xb_d[s, t])
                if unit == 0:
                    dma_l1_weights_rest(0)
                if unit == 1:
                    nc.sync.dma_start(ones_t[:], ones_d)
                    dma_rest_weights(0)
                if t == 0 and s > 0:
                    dma_rest_weights(s)
                if t == NT - 1 and s + 1 < S:
                    dma_l1_weights(s + 1)
                h1 = hpool.tile([128, 10 * T], F32R, tag="h1", name="h1")
                h2 = h2pool.tile([128, 8 * T], DF16, tag="h2", name="h2")
                h3 = h2pool.tile([128, 6 * T], F32R, tag="h3", name="h3")
                l1f = l1_slot_fns(s, xa_t[:].rearrange("p (i q) -> p i q", i=2),
                                  xb_t[:].rearrange("p (i q) -> p i q", i=2), h1)
                s2f = []
                if pending is not None:
                    us_, uu_, h1_, h2_, h3_ = pending
                    s2f = (l2_slot_fns(us_, h1_, h2_) + l3_slot_fns(us_, h2_, h3_)
                           + [readout_fn(us_, uu_, h3_)])
                # interleave ~1 L1 slot per 2 stage-2 slots, L1 first
                if s2f:
                    order = [l1f[0], s2f[0], s2f[1], l1f[1], s2f[2], s2f[3],
                             l1f[2], s2f[4], s2f[5], s2f[6],
                             l1f[3], l1f[4], s2f[7]]
                else:
                    order = l1f
                for fn in order:
                    fn()
                pending = (s, unit, h1, h2, h3)
            us_, uu_, h1_, h2_, h3_ = pending
            for fn in (l2_slot_fns(us_, h1_, h2_) + l3_slot_fns(us_, h2_, h3_)
                       + [readout_fn(us_, uu_, h3_)]):
                fn()

            nc.sync.dma_start(acc_d, acc_t[:])

    nc.compile()
    _BUILD_CACHE[C] = nc
    return nc


# ----------------------------------------------------------------------------
# host-side packing
# ----------------------------------------------------------------------------
def _celu64(x):
    return np.where(x > 0, x, ALPHA * np.expm1(np.minimum(x, 0) / ALPHA))


def _q(x, dt):
    return np.asarray(x).astype(dt).astype(np.float64)


def prep_inputs(species, aev, W1, b1, W2, b2, W3, b3, W4, b4):
    sp = np.asarray(species).reshape(-1)
    n_atoms = sp.shape[0]
    aev0 = np.asarray(aev, dtype=np.float32).reshape(n_atoms, F0)
    W1, b1, W2, b2, W3, b3, W4, b4 = [np.asarray(a, np.float64) for a in
                                      (W1, b1, W2, b2, W3, b3, W4, b4)]

    order = np.argsort(sp, kind="stable")
    cnt = np.bincount(sp.astype(np.int64), minlength=S)
    starts = np.concatenate([[0], np.cumsum(cnt)])
    # device capacity: largest tile-multiple that every species fills exactly
    # (no padding on device); overflow atoms are evaluated on the host in f64
    C = min(C_MAX, (int(cnt.min()) // (N_CORES * T)) * T)
    assert C >= T, "species too unbalanced for device path"
    NT = C // T
    N_UNITS = S * NT
    dev_cnt = np.minimum(cnt, N_CORES * C)

    aev8 = aev0.astype(E4M3).astype(np.float32)

    xas = [np.zeros((S, NT, 128, 2 * T), dtype=E4M3) for _ in range(N_CORES)]
    xbs = [np.zeros((S, NT, 65, 2 * T), dtype=E4M3) for _ in range(N_CORES)]
    for s in range(S):
        idx = order[starts[s]:starts[s] + dev_cnt[s]]
        blk = aev8[idx]
        for c in range(N_CORES):
            seg = blk[c * C:(c + 1) * C]
            xf = np.concatenate([seg.T, np.ones((2, C), np.float32)], axis=0)
            for t in range(NT):
                col = xf[:, t * T:(t + 1) * T]
                xas[c][s, t] = col[0:256].reshape(128, 2 * T).astype(E4M3)
                xbs[c][s, t] = col[256:386].reshape(65, 2 * T).astype(E4M3)

    # W1 DR-packed with bias + bias-residual rows (fp8)
    w1a = np.zeros((S, 128, 10 * 256), dtype=E4M3)
    w1b = np.zeros((S, 65, 10 * 256), dtype=E4M3)
    for s in range(S):
        cols = np.zeros((F0, 1280), np.float64)
        brow = np.zeros(1280, np.float64)
        for c in range(8):
            cols[:, 128 * c:128 * (c + 1)] = W1[s, c, :, 0:128]
            brow[128 * c:128 * (c + 1)] = b1[s, c, 0, 0:128] + ALPHA
        for r in range(2):
            for j in range(4):
                c0 = 1024 + 128 * r + 32 * j
                cols[:, c0:c0 + 32] = W1[s, 4 * r + j, :, 128:160]
                brow[c0:c0 + 32] = b1[s, 4 * r + j, 0, 128:160] + ALPHA
        b_hi = _q(brow, E4M3)
        b_lo = brow - b_hi
        full = np.concatenate([cols, b_hi[None, :], b_lo[None, :]], axis=0)
        fq = full.astype(np.float32).astype(E4M3)
        for k, c in enumerate(CHUNK_ORDER):
            blkc = fq[:, 128 * c:128 * (c + 1)].astype(np.float32)
            w1a[s][:, k * 256:(k + 1) * 256] = blkc[0:256].reshape(128, 256).astype(E4M3)
            w1b[s][:, k * 256:(k + 1) * 256] = blkc[256:386].reshape(65, 256).astype(E4M3)

    w2m = np.zeros((S, 128, M * 128), np.float32)
    w2r = np.zeros((S, 128, 2 * 128), np.float32)
    bl2 = np.zeros((S, 128, 256), np.float32)
    for s in range(S):
        for m in range(M):
            reg, j = m // 4, m % 4
            w2m[s][:, m * 128:(m + 1) * 128] = W2[s, m, 0:128]
            w2r[s][32 * j:32 * (j + 1), reg * 128:(reg + 1) * 128] = W2[s, m, 128:160]
            beff = b2[s, m, 0, :] - ALPHA * W2[s, m].sum(axis=0) + ALPHA
            bl2[s][32 * j, reg * 128:reg * 128 + 128] = beff.astype(np.float32)

    w3p = np.zeros((S, 128, M * 96), dtype=np.float16)
    bl3 = np.zeros((S, 128, 256), np.float32)
    w4p = np.zeros((S, 128, 6), np.float32)
    for s in range(S):
        W3q = _q(W3[s].astype(np.float32), np.float16)     # device-quantized W3
        beff3 = np.stack([b3[s, m, 0, :] - ALPHA * W3q[m].sum(axis=0) + ALPHA
                          for m in range(M)])          # [M, 96]
        for pi, (m, f0, f1, bank, o) in enumerate(L3_PIECES):
            w = f1 - f0
            w3p[s][:, L3_PCOL[pi]:L3_PCOL[pi] + w] = W3q[m, :, f0:f1].astype(np.float16)
            j, blk = bank % 4, bank // 4
            bl3[s][32 * j, blk * 128 + o:blk * 128 + o + w] = \
                beff3[m, f0:f1].astype(np.float32)
            w4p[s][o:o + w, bank] = W4[s, m, f0:f1, 0].astype(np.float32)

    common = {"w1a": w1a, "w1b": w1b, "w2m": w2m, "w2r": w2r, "w3": w3p,
              "w4": w4p, "bl2": bl2, "bl3": bl3,
              "ones": np.ones((128, T), np.float32)}
    in_maps = [dict(common, xa=xas[c], xb=xbs[c]) for c in range(N_CORES)]

    w4sum = np.array([_q(W4[s, :, :, 0].astype(np.float32), np.float64).sum()
                      for s in range(S)])
    b4sum = b4[:, :, 0, 0].sum(axis=1)

    leftover = 0.0
    for s in range(S):
        n_left = int(cnt[s] - dev_cnt[s])
        if n_left <= 0:
            continue
        idx = order[starts[s] + dev_cnt[s]:starts[s + 1]]
        x = aev0[idx].astype(np.float64)
        for m in range(M):
            h = _celu64(x @ W1[s, m] + b1[s, m, 0])
            h = _celu64(h @ W2[s, m] + b2[s, m, 0])
            h = _celu64(h @ W3[s, m] + b3[s, m, 0])
            leftover += float((h @ W4[s, m, :, 0]).sum()) + n_left * float(b4[s, m, 0, 0])

    def finish(results):
        tot = 0.0
        for res in results:
            a = res["acc"].astype(np.float64).reshape(N_UNITS)
            for s in range(S):
                for t in range(NT):
                    tot += a[s * NT + t] - T * ALPHA * w4sum[s]
        for s in range(S):
            tot += dev_cnt[s] * b4sum[s]
        tot += leftover
        return np.array([tot / M], dtype=np.float32)

    return C, in_maps, finish


def _ensure_axon_platform():
    try:
        import jax
        devs = jax.devices()
        if len(devs) >= N_CORES and devs[0].platform != "cpu":
            return
        jax.config.update("jax_platforms", "axon")
    except Exception:
        pass


def kernel(**inputs):
    from concourse.bass_utils import run_bass_kernel_spmd
    _ensure_axon_platform()
    C, in_maps, finish = prep_inputs(**inputs)
    nc = build_kernel(C)
    res = run_bass_kernel_spmd(nc, in_maps, list(range(N_CORES)))
    return finish(res.results)


# revision 8
# speedup vs baseline: 1.0739x; 1.0109x over previous
"""Trainium2 Bass kernel for nn_BmmEnsemble (species-routed CELU-MLP ensemble).

Strategy (data-parallel over atoms, 8 NeuronCores):
  host: stable-sort atoms by species, shard species blocks across cores
        (C=1536 atoms/species/core, no padding), fp8-quantize aev,
        pack DoubleRow operands, pre-fold biases.
  device per tile-unit (species s, 512-atom tile t):
    L1: z+b+a in PSUM via fp8e4m3 DoubleRow matmuls (2 per 128-out chunk,
        K=386 = 384 aev rows + fp8 bias row + fp8 bias-residual row).
    drain (exact CELU, 2 passes):  celu(w)+a == max(w+a, min(a*e^{10 w}, a))
        ACT:  u' = Exp(10*psum + (ln a - 10 a)) -> bf16    [psum holds w+a]
        DVE:  h = (u' MIN a) MAX psum  (one scalar_tensor_tensor)
        +a shift folded into next layer's bias: b_eff = b - a*sum_fin(Wq) + a.
    L2: f32r (exact) weights x f32r h1; L3: fp16 W3 x fp16 h2, with the
        8 models' 96-wide outputs packed into 6 full 128-partition PSUM
        banks (pieces at 32-aligned tile positions) so drains touch 6
        banks instead of 8; f32r bias matmuls (one per bank).
    readout: 6 accumulating W4 (f32r, bank-concatenated) matmuls ->
        e[1, 512] PSUM row, ACT Identity accum -> acc column per unit.
  PSUM: 2-bank slots rotating 4-deep; emission interleaves unit u+1's L1
  slots through unit u's L2/L3/readout so no engine waits on drains.
  Host: subtract a*sum(W4) shift terms, add b4 terms, evaluate the
  4*212 leftover atoms exactly in f64.
"""
import math
import numpy as np
import ml_dtypes

BF16 = ml_dtypes.bfloat16
E4M3 = ml_dtypes.float8_e4m3fn if hasattr(ml_dtypes, 'float8_e4m3fn') else ml_dtypes.float8_e4m3

S = 4
M = 8
F0, F1, F2, F3 = 384, 160, 128, 96
ALPHA = 0.1
N_CORES = 8
T = 512
C_MAX = 1536
LNAC = math.log(ALPHA) - 10.0 * ALPHA

# h1 tile block layout: drain-order blocks; chunk -> block
#   slot1 = chunks [8,9,0,1] -> blocks 0..3
#   slot2 = chunks [2,3,4,5] -> blocks 4..7
#   slot3 = chunks [6,7]     -> blocks 8..9
CHUNK_ORDER = [8, 9, 0, 1, 2, 3, 4, 5, 6, 7]
BLK = {c: i for i, c in enumerate(CHUNK_ORDER)}

# L3 output packing: 8 models x 96 feats -> 6 full 128-partition banks.
# Pieces per 4-model group g (base model 4g, base bank 3g):
#   (model, f0, f1, bank, partition offset)
L3_PIECES = []
for _g in (0, 1):
    _M, _B = 4 * _g, 3 * _g
    L3_PIECES += [
        (_M + 0, 0, 96, _B + 0, 0),
        (_M + 1, 0, 32, _B + 0, 96),
        (_M + 1, 32, 96, _B + 1, 0),
        (_M + 2, 0, 64, _B + 1, 64),
        (_M + 2, 64, 96, _B + 2, 0),
        (_M + 3, 0, 32, _B + 2, 32),
        (_M + 3, 32, 64, _B + 2, 64),
        (_M + 3, 64, 96, _B + 2, 96),
    ]
# piece -> column offset in the packed w3 tile (piece-major)
L3_PCOL = []
_off = 0
for (_m, _f0, _f1, _b, _o) in L3_PIECES:
    L3_PCOL.append(_off)
    _off += _f1 - _f0

_BUILD_CACHE = {}


def build_kernel(C=C_MAX):
    if C in _BUILD_CACHE:
        return _BUILD_CACHE[C]
    NT = C // T
    N_UNITS = S * NT

    import concourse.bacc as bacc
    import concourse.tile as tile
    import concourse.mybir as mybir

    F32 = mybir.dt.float32
    F32R = mybir.dt.float32r
    DBF = mybir.dt.bfloat16
    DF16 = mybir.dt.float16
    DE4 = mybir.dt.float8e4
    MIN, MAX, ADD = mybir.AluOpType.min, mybir.AluOpType.max, mybir.AluOpType.add
    EXP = mybir.ActivationFunctionType.Exp
    RELU = mybir.ActivationFunctionType.Relu
    DR = mybir.MatmulPerfMode.DoubleRow
    BANK = 512

    nc = bacc.Bacc("TRN2", target_bir_lowering=False, debug=False)

    xa_d = nc.dram_tensor("xa", [S, NT, 128, 2 * T], DE4, kind="ExternalInput").ap()
    xb_d = nc.dram_tensor("xb", [S, NT, 65, 2 * T], DE4, kind="ExternalInput").ap()
    w1a_d = nc.dram_tensor("w1a", [S, 128, 10 * 256], DE4, kind="ExternalInput").ap()
    w1b_d = nc.dram_tensor("w1b", [S, 65, 10 * 256], DE4, kind="ExternalInput").ap()
    w2m_d = nc.dram_tensor("w2m", [S, 128, M * 128], F32R, kind="ExternalInput").ap()
    w2r_d = nc.dram_tensor("w2r", [S, 128, 2 * 128], F32R, kind="ExternalInput").ap()
    w3_d = nc.dram_tensor("w3", [S, 128, M * 96], DF16, kind="ExternalInput").ap()
    w4_d = nc.dram_tensor("w4", [S, 128, 6], F32R, kind="ExternalInput").ap()
    bl2_d = nc.dram_tensor("bl2", [S, 128, 256], F32R, kind="ExternalInput").ap()
    bl3_d = nc.dram_tensor("bl3", [S, 128, 256], F32R, kind="ExternalInput").ap()
    ones_d = nc.dram_tensor("ones", [128, T], F32R, kind="ExternalInput").ap()
    acc_d = nc.dram_tensor("acc", [1, N_UNITS], F32, kind="ExternalOutput").ap()

    with tile.TileContext(nc) as tc:
        with tc.tile_pool(name="wpool", bufs=1) as wpool, \
             tc.tile_pool(name="xpool", bufs=2) as xpool, \
             tc.tile_pool(name="hpool", bufs=2) as hpool, \
             tc.tile_pool(name="h2pool", bufs=1) as h2pool, \
             tc.tile_pool(name="upool", bufs=3) as upool, \
             tc.tile_pool(name="apool", bufs=1) as apool, \
             tc.tile_pool(name="ps", bufs=4, space="PSUM") as psp:

            w1a_t, w1b_t, w2m_t, w2r_t, w3_t, w4_t, bl2_t, bl3_t = ({} for _ in range(8))
            for s in range(S):
                w1a_t[s] = wpool.tile([128, 10 * 256], DE4, tag=f"w1a{s}", name=f"w1a{s}")
                w1b_t[s] = wpool.tile([65, 10 * 256], DE4, tag=f"w1b{s}", name=f"w1b{s}")
                w2m_t[s] = wpool.tile([128, M * 128], F32R, tag=f"w2m{s}", name=f"w2m{s}")
                w2r_t[s] = wpool.tile([128, 2 * 128], F32R, tag=f"w2r{s}", name=f"w2r{s}")
                w3_t[s] = wpool.tile([128, M * 96], DF16, tag=f"w3{s}", name=f"w3{s}")
                w4_t[s] = wpool.tile([128, 6], F32R, tag=f"w4{s}", name=f"w4{s}")
                bl2_t[s] = wpool.tile([128, 256], F32R, tag=f"bl2{s}", name=f"bl2{s}")
                bl3_t[s] = wpool.tile([128, 256], F32R, tag=f"bl3{s}", name=f"bl3{s}")

            def dma_l1_weights(s, split=False):
                nc.sync.dma_start(w1a_t[s][:, 0:4 * 256], w1a_d[s][:, 0:4 * 256])
                nc.sync.dma_start(w1b_t[s][:, 0:4 * 256], w1b_d[s][:, 0:4 * 256])
                if not split:
                    nc.sync.dma_start(w1a_t[s][:, 4 * 256:], w1a_d[s][:, 4 * 256:])
                    nc.sync.dma_start(w1b_t[s][:, 4 * 256:], w1b_d[s][:, 4 * 256:])

            def dma_l1_weights_rest(s):
                nc.sync.dma_start(w1a_t[s][:, 4 * 256:], w1a_d[s][:, 4 * 256:])
                nc.sync.dma_start(w1b_t[s][:, 4 * 256:], w1b_d[s][:, 4 * 256:])

            def dma_rest_weights(s):
                for tt, dd in ((w2m_t[s], w2m_d[s]), (w2r_t[s], w2r_d[s]),
                               (w3_t[s], w3_d[s]), (w4_t[s], w4_d[s]),
                               (bl2_t[s], bl2_d[s]), (bl3_t[s], bl3_d[s])):
                    nc.sync.dma_start(tt[:], dd)

            ones_t = wpool.tile([128, T], F32R, tag="ones", name="ones")
            lnac_t = wpool.tile([128, 1], F32, tag="lnac", name="lnac")
            nc.vector.memset(lnac_t[:], LNAC)
            # warm the ACT Exp table during the initial DMA window
            warm_t = wpool.tile([128, 1], F32, tag="warm", name="warm")
            nc.scalar.activation(warm_t[:], lnac_t[:],
                                 mybir.ActivationFunctionType.Exp,
                                 bias=0.0, scale=1.0)
            acc_t = apool.tile([1, N_UNITS], F32, tag="acc", name="acc")
            junk_t = apool.tile([1, T], F32, tag="junk", name="junk")

            def drain(ps_t, nrow, gsz, h_view):
                ps_v = ps_t[0:nrow, :].rearrange("p (g q) -> p g q", q=BANK)[:, 0:gsz, 0:T]
                u = upool.tile([128, 2 * T], DBF, tag="u", name="u")
                u_v = u[0:nrow, 0:gsz * T].rearrange("p (g q) -> p g q", q=T)
                nc.scalar.activation(u_v, ps_v, EXP, bias=lnac_t[0:nrow, 0:1], scale=10.0)
                nc.vector.scalar_tensor_tensor(h_view, u_v, ALPHA, ps_v, op0=MIN, op1=MAX)

            def hview(h_t, b0, g, nrow=128):
                return h_t[0:nrow, b0 * T:(b0 + g) * T].rearrange("p (g q) -> p g q", q=T)

            def l1_slot_fns(s, xa_v, xb_v, h1):
                def mk(slot_i, chunks):
                    def fn():
                        ps_t = psp.tile([128, 2 * BANK], F32, tag="psg", name=f"l1s{slot_i}")
                        for g, c in enumerate(chunks):
                            k = BLK[c]  # host packs chunk CHUNK_ORDER[k] at col block k
                            nc.tensor.matmul(
                                ps_t[:, g * BANK:g * BANK + T],
                                w1a_t[s][:, k * 256:(k + 1) * 256].rearrange(
                                    "p (i m) -> p i m", i=2),
                                xa_v, start=True, stop=False, perf_mode=DR)
                            nc.tensor.matmul(
                                ps_t[:, g * BANK:g * BANK + T],
                                w1b_t[s][:, k * 256:(k + 1) * 256].rearrange(
                                    "p (i m) -> p i m", i=2),
                                xb_v, start=False, stop=True, perf_mode=DR)
                        drain(ps_t, 128, len(chunks),
                              hview(h1, BLK[chunks[0]], len(chunks)))
                    return fn
                return [mk(i, ch) for i, ch in
                        enumerate(([8, 9], [0, 1], [2, 3], [4, 5], [6, 7]))]

            def l2_slot_fns(s, h1, h2):
                def mk(m0):
                  def fn():
                    ps_t = psp.tile([128, 2 * BANK], F32, tag="psg", name=f"l2m{m0}")
                    for g in range(2):
                        m = m0 + g
                        reg, j = m // 4, m % 4
                        bm = BLK[m]
                        sl = slice(g * BANK, g * BANK + T)
                        nc.tensor.matmul(
                            ps_t[:, sl],
                            w2m_t[s][:, m * 128:(m + 1) * 128],
                            h1[:, bm * T:(bm + 1) * T],
                            start=True, stop=False)
                        nc.tensor.matmul(
                            ps_t[:, sl],
                            w2r_t[s][32 * j:32 * (j + 1), reg * 128:(reg + 1) * 128],
                            h1[32 * j:32 * (j + 1), BLK[8 + reg] * T:(BLK[8 + reg] + 1) * T],
                            start=False, stop=False, tile_position=(32 * j, 0))
                        nc.tensor.matmul(
                            ps_t[0:128, sl],
                            bl2_t[s][32 * j:32 * j + 1, reg * 128:reg * 128 + 128],
                            ones_t[32 * j:32 * j + 1, 0:T],
                            start=False, stop=True, tile_position=(32 * j, 0))
                    drain(ps_t, 128, 2, hview(h2, m0, 2))
                  return fn
                return [mk(m0) for m0 in (0, 2, 4, 6)]

            def l3_slot_fns(s, h2, h3):
                # packed L3: 6 full banks, pieces per L3_PIECES
                def mk(b0):
                  def fn():
                    ps_t = psp.tile([128, 2 * BANK], F32, tag="psg", name=f"l3b{b0}")
                    for g in range(2):
                        bank = b0 + g
                        sl = slice(g * BANK, g * BANK + T)
                        for pi, (m, f0, f1, bk, o) in enumerate(L3_PIECES):
                            if bk != bank:
                                continue
                            w = f1 - f0
                            # pieces cover disjoint partition ranges: each
                            # must open its own accumulation region
                            nc.tensor.matmul(
                                ps_t[o:o + w, sl],
                                w3_t[s][:, L3_PCOL[pi]:L3_PCOL[pi] + w],
                                h2[:, m * T:(m + 1) * T],
                                start=True, stop=False,
                                tile_position=(0, o))
                        j, blk = bank % 4, bank // 4
                        nc.tensor.matmul(
                            ps_t[0:128, sl],
                            bl3_t[s][32 * j:32 * j + 1, blk * 128:blk * 128 + 128],
                            ones_t[32 * j:32 * j + 1, 0:T],
                            start=False, stop=True, tile_position=(32 * j, 0))
                    drain(ps_t, 128, 2, hview(h3, b0, 2))
                  return fn
                return [mk(b0) for b0 in (0, 2, 4)]

            def readout_fn(s, unit, h3):
              def fn():
                ps_t = psp.tile([128, 2 * BANK], F32, tag="psg", name="eps")
                for k in range(6):
                    nc.tensor.matmul(
                        ps_t[0:1, 0:T],
                        w4_t[s][:, k:k + 1],
                        h3[:, k * T:(k + 1) * T],
                        start=(k == 0), stop=(k == 5))
                nc.scalar.activation(
                    junk_t[:], ps_t[0:1, 0:T],
                    mybir.ActivationFunctionType.Identity,
                    bias=0.0, scale=1.0,
                    accum_out=acc_t[0:1, unit:unit + 1])
              return fn

            units = [(s, t) for s in range(S) for t in range(NT)]
            pending = None
            dma_l1_weights(0, split=True)
            for unit, (s, t) in enumerate(units):
                xa_t = xpool.tile([128, 2 * T], DE4, tag="xa", name="xa")
                xb_t = xpool.tile([65, 2 * T], DE4, tag="xb", name="xb")
                nc.sync.dma_start(xa_t[:], xa_d[s, t])
                nc.sync.dma_start(xb_t[:], xb_d[s, t])
                if unit == 0:
                    dma_l1_weights_rest(0)
                if unit == 1:
                    nc.sync.dma_start(ones_t[:], ones_d)
                    dma_rest_weights(0)
                if t == 0 and s > 0:
                    dma_rest_weights(s)
                if t == NT - 1 and s + 1 < S:
                    dma_l1_weights(s + 1)
                h1 = hpool.tile([128, 10 * T], F32R, tag="h1", name="h1")
                h2 = h2pool.tile([128, 8 * T], DF16, tag="h2", name="h2")
                h3 = h2pool.tile([128, 6 * T], F32R, tag="h3", name="h3")
                l1f = l1_slot_fns(s, xa_t[:].rearrange("p (i q) -> p i q", i=2),
                                  xb_t[:].rearrange("p (i q) -> p i q", i=2), h1)
                s2f = []
                if pending is not None:
                    us_, uu_, h1_, h2_, h3_ = pending
                    s2f = (l2_slot_fns(us_, h1_, h2_) + l3_slot_fns(us_, h2_, h3_)
                           + [readout_fn(us_, uu_, h3_)])
                # interleave ~1 L1 slot per 2 stage-2 slots, L1 first
                if s2f:
                    order = [l1f[0], s2f[0], s2f[1], l1f[1], s2f[2], s2f[3],
                             s2f[4], l1f[2], s2f[5], s2f[6],
                             l1f[3], l1f[4], s2f[7]]
                else:
                    order = l1f
                for fn in order:
                    fn()
                pending = (s, unit, h1, h2, h3)
            us_, uu_, h1_, h2_, h3_ = pending
            for fn in (l2_slot_fns(us_, h1_, h2_) + l3_slot_fns(us_, h2_, h3_)
                       + [readout_fn(us_, uu_, h3_)]):
                fn()

            nc.sync.dma_start(acc_d, acc_t[:])

    nc.compile()
    _BUILD_CACHE[C] = nc
    return nc


# ----------------------------------------------------------------------------
# host-side packing
# ----------------------------------------------------------------------------
def _celu64(x):
    return np.where(x > 0, x, ALPHA * np.expm1(np.minimum(x, 0) / ALPHA))


def _q(x, dt):
    return np.asarray(x).astype(dt).astype(np.float64)


def prep_inputs(species, aev, W1, b1, W2, b2, W3, b3, W4, b4):
    sp = np.asarray(species).reshape(-1)
    n_atoms = sp.shape[0]
    aev0 = np.asarray(aev, dtype=np.float32).reshape(n_atoms, F0)
    W1, b1, W2, b2, W3, b3, W4, b4 = [np.asarray(a, np.float64) for a in
                                      (W1, b1, W2, b2, W3, b3, W4, b4)]

    order = np.argsort(sp, kind="stable")
    cnt = np.bincount(sp.astype(np.int64), minlength=S)
    starts = np.concatenate([[0], np.cumsum(cnt)])
    # device capacity: largest tile-multiple that every species fills exactly
    # (no padding on device); overflow atoms are evaluated on the host in f64
    C = min(C_MAX, (int(cnt.min()) // (N_CORES * T)) * T)
    assert C >= T, "species too unbalanced for device path"
    NT = C // T
    N_UNITS = S * NT
    dev_cnt = np.minimum(cnt, N_CORES * C)

    aev8 = aev0.astype(E4M3).astype(np.float32)

    xas = [np.zeros((S, NT, 128, 2 * T), dtype=E4M3) for _ in range(N_CORES)]
    xbs = [np.zeros((S, NT, 65, 2 * T), dtype=E4M3) for _ in range(N_CORES)]
    for s in range(S):
        idx = order[starts[s]:starts[s] + dev_cnt[s]]
        blk = aev8[idx]
        for c in range(N_CORES):
            seg = blk[c * C:(c + 1) * C]
            xf = np.concatenate([seg.T, np.ones((2, C), np.float32)], axis=0)
            for t in range(NT):
                col = xf[:, t * T:(t + 1) * T]
                xas[c][s, t] = col[0:256].reshape(128, 2 * T).astype(E4M3)
                xbs[c][s, t] = col[256:386].reshape(65, 2 * T).astype(E4M3)

    # W1 DR-packed with bias + bias-residual rows (fp8)
    w1a = np.zeros((S, 128, 10 * 256), dtype=E4M3)
    w1b = np.zeros((S, 65, 10 * 256), dtype=E4M3)
    for s in range(S):
        cols = np.zeros((F0, 1280), np.float64)
        brow = np.zeros(1280, np.float64)
        for c in range(8):
            cols[:, 128 * c:128 * (c + 1)] = W1[s, c, :, 0:128]
            brow[128 * c:128 * (c + 1)] = b1[s, c, 0, 0:128] + ALPHA
        for r in range(2):
            for j in range(4):
                c0 = 1024 + 128 * r + 32 * j
                cols[:, c0:c0 + 32] = W1[s, 4 * r + j, :, 128:160]
                brow[c0:c0 + 32] = b1[s, 4 * r + j, 0, 128:160] + ALPHA
        b_hi = _q(brow, E4M3)
        b_lo = brow - b_hi
        full = np.concatenate([cols, b_hi[None, :], b_lo[None, :]], axis=0)
        fq = full.astype(np.float32).astype(E4M3)
        for k, c in enumerate(CHUNK_ORDER):
            blkc = fq[:, 128 * c:128 * (c + 1)].astype(np.float32)
            w1a[s][:, k * 256:(k + 1) * 256] = blkc[0:256].reshape(128, 256).astype(E4M3)
            w1b[s][:, k * 256:(k + 1) * 256] = blkc[256:386].reshape(65, 256).astype(E4M3)

    w2m = np.zeros((S, 128, M * 128), np.float32)
    w2r = np.zeros((S, 128, 2 * 128), np.float32)
    bl2 = np.zeros((S, 128, 256), np.float32)
    for s in range(S):
        for m in range(M):
            reg, j = m // 4, m % 4
            w2m[s][:, m * 128:(m + 1) * 128] = W2[s, m, 0:128]
            w2r[s][32 * j:32 * (j + 1), reg * 128:(reg + 1) * 128] = W2[s, m, 128:160]
            beff = b2[s, m, 0, :] - ALPHA * W2[s, m].sum(axis=0) + ALPHA
            bl2[s][32 * j, reg * 128:reg * 128 + 128] = beff.astype(np.float32)

    w3p = np.zeros((S, 128, M * 96), dtype=np.float16)
    bl3 = np.zeros((S, 128, 256), np.float32)
    w4p = np.zeros((S, 128, 6), np.float32)
    for s in range(S):
        W3q = _q(W3[s].astype(np.float32), np.float16)     # device-quantized W3
        beff3 = np.stack([b3[s, m, 0, :] - ALPHA * W3q[m].sum(axis=0) + ALPHA
                          for m in range(M)])          # [M, 96]
        for pi, (m, f0, f1, bank, o) in enumerate(L3_PIECES):
            w = f1 - f0
            w3p[s][:, L3_PCOL[pi]:L3_PCOL[pi] + w] = W3q[m, :, f0:f1].astype(np.float16)
            j, blk = bank % 4, bank // 4
            bl3[s][32 * j, blk * 128 + o:blk * 128 + o + w] = \
                beff3[m, f0:f1].astype(np.float32)
            w4p[s][o:o + w, bank] = W4[s, m, f0:f1, 0].astype(np.float32)

    common = {"w1a": w1a, "w1b": w1b, "w2m": w2m, "w2r": w2r, "w3": w3p,
              "w4": w4p, "bl2": bl2, "bl3": bl3,
              "ones": np.ones((128, T), np.float32)}
    in_maps = [dict(common, xa=xas[c], xb=xbs[c]) for c in range(N_CORES)]

    w4sum = np.array([_q(W4[s, :, :, 0].astype(np.float32), np.float64).sum()
                      for s in range(S)])
    b4sum = b4[:, :, 0, 0].sum(axis=1)

    leftover = 0.0
    for s in range(S):
        n_left = int(cnt[s] - dev_cnt[s])
        if n_left <= 0:
            continue
        idx = order[starts[s] + dev_cnt[s]:starts[s + 1]]
        x = aev0[idx].astype(np.float64)
        for m in range(M):
            h = _celu64(x @ W1[s, m] + b1[s, m, 0])
            h = _celu64(h @ W2[s, m] + b2[s, m, 0])
            h = _celu64(h @ W3[s, m] + b3[s, m, 0])
            leftover += float((h @ W4[s, m, :, 0]).sum()) + n_left * float(b4[s, m, 0, 0])

    def finish(results):
        tot = 0.0
        for res in results:
            a = res["acc"].astype(np.float64).reshape(N_UNITS)
            for s in range(S):
                for t in range(NT):
                    tot += a[s * NT + t] - T * ALPHA * w4sum[s]
        for s in range(S):
            tot += dev_cnt[s] * b4sum[s]
        tot += leftover
        return np.array([tot / M], dtype=np.float32)

    return C, in_maps, finish


def _ensure_axon_platform():
    try:
        import jax
        devs = jax.devices()
        if len(devs) >= N_CORES and devs[0].platform != "cpu":
            return
        jax.config.update("jax_platforms", "axon")
    except Exception:
        pass


def kernel(**inputs):
    from concourse.bass_utils import run_bass_kernel_spmd
    _ensure_axon_platform()
    C, in_maps, finish = prep_inputs(**inputs)
    nc = build_kernel(C)
    res = run_bass_kernel_spmd(nc, in_maps, list(range(N_CORES)))
    return finish(res.results)
